# revision 41
# baseline (speedup 1.0000x reference)
"""Trainium2 Bass kernel for nn_BlockLayer_75376676045426 (gnn_message_passing).

Math (N=2048 nodes, E=67584 edges, F=1024 features, 8 NeuronCores):
  L = I - D^-1/2 A D^-1/2,  S = D^-1/2 A D^-1/2.  The reference's
  eigh-based wavelet weights are analytic functions of S:
      w1 = exp(-2L) = g(S),   w2 = exp(-4 exp(-2L)) = f(S).
  S has the Perron pair (lambda=1, u = sqrt(d)/||sqrt(d)||) in closed form;
  after deflating it exactly, the rest of the spectrum sits inside
  [-0.4, 0.4], so w1@h, w2@h are evaluated with a single shared degree-3
  Chebyshev recurrence.
  r = h@W1 + (w1 h)@W2 + (w2 h)@W3 + bias;  then GAT-style edge softmax:
  logits_e = alpha[src] + beta[dst] + gamma_e; segment softmax over dst;
  out = P@z + rank-2 term, with the dense attention matrix P built on-chip
  via gpsimd local_scatter (multi-edge duplicates go to overflow columns).

Sharding: phase A column-parallel (adj replicated in SBUF fp16, h columns
split 8 ways); the A2A payload is written pre-transposed so phase B is
pure matmuls; phase B h@W1 part runs during the A2A; edge-phase scatter
runs during the Chebyshev recurrence; softmax overlaps the z AllGather.
"""

import sys

sys.path.insert(0, "/opt/trn_rl_repo")

import numpy as np
from numpy.polynomial import chebyshev as _cheb

import concourse.bacc as bacc
import concourse.bass as bass
import concourse.mybir as mybir
import concourse.tile as tile
from concourse.bass_utils import run_bass_kernel_spmd
from concourse.masks import make_identity

P = 128
N = 2048
F = 1024
C = 8            # cores
R = N // C       # dst rows per core (256)
NT = N // P      # 16 node tiles
KT = F // P      # 8 feature tiles
COLS = F // C    # 128 h-columns per core
B_CHEB = 0.37    # Chebyshev half-width for the bulk spectrum of S
DEG = 2
BIG = 30000.0

fp16 = mybir.dt.float16
f32 = mybir.dt.float32
i16 = mybir.dt.int16
i32 = mybir.dt.int32
AF = mybir.ActivationFunctionType
ALU = mybir.AluOpType
ts = bass.ts


def _cheb_coeffs():
    g = lambda y: np.exp(-2.0 * (1.0 - B_CHEB * y))
    f = lambda y: np.exp(-4.0 * np.exp(-2.0 * (1.0 - B_CHEB * y)))
    return (_cheb.chebinterpolate(g, DEG).astype(np.float64),
            _cheb.chebinterpolate(f, DEG).astype(np.float64))


def _host_prep(e, src, dst):
    """Index/layout-only host prep: stable sort by (dst, src), padded
    per-row scatter layouts, overflow slots for duplicate (dst, src) cells."""
    src = np.asarray(src).astype(np.int64)
    dst = np.asarray(dst).astype(np.int64)
    e = np.asarray(e)
    E = src.shape[0]
    order = np.lexsort((src, dst))
    ds, ss = dst[order], src[order]
    eo = np.ascontiguousarray(e[order])

    cell = ds * N + ss
    first = np.r_[True, cell[1:] != cell[:-1]]
    idxs = np.arange(E)
    ranks = idxs - np.maximum.accumulate(np.where(first, idxs, 0))

    l0 = ranks == 0
    J0 = 0
    for hf in (0, 1):
        sel = l0 & ((ss // 1024) == hf)
        J0 = max(J0, int(np.bincount(ds[sel], minlength=N).max()))
    J0 = (J0 + 1) // 2 * 2
    halves = []
    for hf in (0, 1):
        sel = np.where(l0 & ((ss // 1024) == hf))[0]
        idx_arr = np.full((N, J0), -1, np.int16)
        e0_arr = np.zeros((N, J0), np.float32)
        e1_arr = np.zeros((N, J0), np.float32)
        pos = np.zeros(N, np.int64)
        for k in sel:
            n = ds[k]
            j = pos[n]; pos[n] = j + 1
            idx_arr[n, j] = ss[k] - 1024 * hf
            e0_arr[n, j] = eo[k, 0]
            e1_arr[n, j] = eo[k, 1]
        halves.append((idx_arr, e0_arr, e1_arr))

    ov = np.where(ranks >= 1)[0]
    J_OV = max(2, int(np.bincount(ds[ov], minlength=N).max()) if len(ov) else 2)
    J_OV = (J_OV + 1) // 2 * 2
    e0o = np.zeros((N, J_OV), np.float32)
    e1o = np.zeros((N, J_OV), np.float32)
    mo = np.zeros((N, J_OV), np.float32)
    aoff = np.zeros((N, J_OV), np.int32)
    zoff = np.zeros((N, J_OV), np.int32)
    pos = np.zeros(N, np.int64)
    for k in ov:
        n = ds[k]
        j = pos[n]; pos[n] = j + 1
        e0o[n, j] = eo[k, 0]
        e1o[n, j] = eo[k, 1]
        mo[n, j] = 1.0
        s = int(ss[k])
        aoff[n, j] = s
        zoff[n, j] = s
    return halves, J0, (e0o, e1o, mo, aoff, zoff), J_OV


def _build_program(J0, J_OV):
    cg, cf = _cheb_coeffs()
    W = N + ((J_OV + 7) // 8) * 8
    nc = bacc.Bacc("TRN2", target_bir_lowering=False, debug=False, num_devices=C)

    # ---------------- DRAM I/O ----------------
    d_adj = nc.dram_tensor("adj", [N, N], fp16, kind="ExternalInput").ap()
    d_hcol = nc.dram_tensor("hcol", [N, COLS], fp16, kind="ExternalInput").ap()
    d_hcolT = nc.dram_tensor("hcolT", [COLS, N], fp16, kind="ExternalInput").ap()
    d_wsl = nc.dram_tensor("wsl", [3 * P, F], fp16, kind="ExternalInput").ap()
    d_hrowT = nc.dram_tensor("hrowT", [F, R], fp16, kind="ExternalInput").ap()
    d_w = [nc.dram_tensor(f"w{i}", [F, F], fp16, kind="ExternalInput").ap()
           for i in (1, 2, 3)]
    d_bias = nc.dram_tensor("biasv", [1, F], f32, kind="ExternalInput").ap()
    d_attnw = nc.dram_tensor("attnw", [1, 2 * F + 2], f32, kind="ExternalInput").ap()
    d_edgew = nc.dram_tensor("edgew", [2, 2], f32, kind="ExternalInput").ap()
    d_e2nw = nc.dram_tensor("e2nw", [F, 2], f32, kind="ExternalInput").ap()
    d_idx0 = [nc.dram_tensor(f"idx0{hf}", [R, J0], i16, kind="ExternalInput").ap()
              for hf in (0, 1)]
    d_e0h = [nc.dram_tensor(f"e0h{hf}", [R, J0], fp16, kind="ExternalInput").ap()
             for hf in (0, 1)]
    d_e1h = [nc.dram_tensor(f"e1h{hf}", [R, J0], fp16, kind="ExternalInput").ap()
             for hf in (0, 1)]
    d_e0o = nc.dram_tensor("e0o", [R, J_OV], fp16, kind="ExternalInput").ap()
    d_e1o = nc.dram_tensor("e1o", [R, J_OV], fp16, kind="ExternalInput").ap()
    d_mo = nc.dram_tensor("mo", [R, J_OV], fp16, kind="ExternalInput").ap()
    d_aoff = nc.dram_tensor("aoff", [R, J_OV], i32, kind="ExternalInput").ap()
    d_zoff = nc.dram_tensor("zoff", [R, J_OV], i32, kind="ExternalInput").ap()
    d_out = nc.dram_tensor("out_rows", [R, F], f32, kind="ExternalOutput").ap()

    # internal DRAM (collective bounce buffers); one A2A carries both y's
    y12T = nc.dram_tensor("y12T", [N, R], fp16).ap()
    y12x = nc.dram_tensor("y12x", [N, R], fp16).ap()
    z_slice = nc.dram_tensor("z_slice", [R, F], fp16).ap()
    zg = nc.dram_tensor("zg", [N, F], fp16, addr_space="Shared").ap()
    ab_part = nc.dram_tensor("ab_part", [N, 1], f32).ap()
    abg = nc.dram_tensor("abg", [N, 1], f32, addr_space="Shared").ap()
    rgroups = [list(range(C))]

    with tile.TileContext(nc) as tc, tc.tile_pool(name="const", bufs=1) as cpool:
        ident = cpool.tile([P, P], fp16)
        make_identity(nc, ident[:])
        id32 = cpool.tile([P, P], f32)
        make_identity(nc, id32[:])
        ones_c16 = cpool.tile([P, 1], fp16)
        nc.vector.memset(ones_c16[:], 1.0)
        ones_r16 = cpool.tile([1, P], fp16)
        nc.vector.memset(ones_r16[:], 1.0)
        ones_r32 = cpool.tile([1, P], f32)
        nc.vector.memset(ones_r32[:], 1.0)
        ones_c32 = cpool.tile([P, 1], f32)
        nc.vector.memset(ones_c32[:], 1.0)
        ones_scat = cpool.tile([P, J0], fp16)
        nc.vector.memset(ones_scat[:], 1.0)
        bias16 = cpool.tile([1, F], fp16)
        nc.gpsimd.dma_start(out=bias16[:], in_=d_bias[:1, :])
        a1_16 = cpool.tile([1, F], fp16)
        nc.gpsimd.dma_start(out=a1_16[:], in_=d_attnw[:1, 0:F])
        a2_16 = cpool.tile([1, F], fp16)
        nc.gpsimd.dma_start(out=a2_16[:], in_=d_attnw[:1, F:2 * F])
        a1B = cpool.tile([P, F], fp16)
        a2B = cpool.tile([P, F], fp16)
        e2nT = cpool.tile([2, F], fp16)
        wa = cpool.tile([P, 4], fp16)
        biasa8 = cpool.tile([1, 2], fp16)
        edgew_sb = cpool.tile([2, 2], f32, tag="edgew")
        nc.sync.dma_start(out=edgew_sb[:2, :], in_=d_edgew[:, :])
        a3_sb = cpool.tile([2, 1], f32, tag="a3")
        nc.sync.dma_start(out=a3_sb[:2, :1], in_=d_attnw[:1, 2 * F:2 * F + 2])
        ew_row = cpool.tile([1, 4], f32, tag="ew_row")
        nc.sync.dma_start(out=ew_row[:1, :], in_=d_edgew[:, :])
        v_row = cpool.tile([1, 2], f32, tag="vrow")
        v01b = cpool.tile([P, 2], f32, tag="v01b")
        ewb = cpool.tile([P, 4], f32, tag="ewb")
        # per-core degree-derived scalars (persist across phases)
        dsum = cpool.tile([P, NT], f32)
        dinv2 = cpool.tile([P, NT], f32)
        dinv = cpool.tile([P, NT], f32)
        sqd = cpool.tile([P, NT], f32)
        dinv2b = cpool.tile([P, NT], f32)

        # ---- startup const broadcasts (PE idle here) ----
        with (
            tc.tile_pool(name="pre", bufs=1) as prep,
            tc.tile_pool(name="ps_pre", bufs=1, space="PSUM") as ps_pre,
        ):
            # v_row = a3^T @ edge_w  [1, 2]
            ps_v = ps_pre.tile([P, 2], f32, space="PSUM", tag="sm")
            nc.tensor.matmul(ps_v[:1, :2], a3_sb[:2, :1], edgew_sb[:2, :],
                             start=True, stop=True)
            nc.vector.tensor_copy(v_row[:1, :2], ps_v[:1, :2])
            ps_b1 = ps_pre.tile([P, 2], f32, space="PSUM", tag="sm")
            nc.tensor.matmul(ps_b1[:, :2], ones_r32[:1, :], v_row[:1, :2],
                             start=True, stop=True)
            nc.vector.tensor_copy(v01b[:], ps_b1[:, :2])
            ps_b2 = ps_pre.tile([P, 4], f32, space="PSUM", tag="sm")
            nc.tensor.matmul(ps_b2[:, :4], ones_r32[:1, :], ew_row[:1, :],
                             start=True, stop=True)
            nc.vector.tensor_copy(ewb[:], ps_b2[:, :4])
            # e2nT [2, F]
            for k in range(KT):
                etile = prep.tile([P, 2], fp16, tag="e2ntile")
                nc.gpsimd.dma_start(out=etile[:], in_=d_e2nw[ts(k, P), :])
                ps_t = ps_pre.tile([P, P], fp16, space="PSUM", tag="tp")
                nc.tensor.transpose(ps_t[:2, :], etile[:], ident[:])
                nc.vector.tensor_copy(e2nT[:2, ts(k, P)], ps_t[:2, :])
            # a1B/a2B broadcasts [P, F]
            for (srcv, dstv) in ((a1_16, a1B), (a2_16, a2B)):
                for chunk in range(2):
                    ps_bb = ps_pre.tile([P, 512], f32, space="PSUM", tag="bc")
                    nc.tensor.matmul(ps_bb[:], ones_r16[:1, :],
                                     srcv[:1, ts(chunk, 512)],
                                     start=True, stop=True)
                    nc.scalar.activation(dstv[:, ts(chunk, 512)],
                                         ps_bb[:], AF.Copy)

        with tc.tile_pool(name="eprep", bufs=1) as eprep:
            # edge-prep tiles that persist into the edge phase
            E0s = [eprep.tile([P, W], fp16, name=f"E0s_{b}", tag=f"E0s_{b}") for b in range(2)]
            E1s = [eprep.tile([P, W], fp16, name=f"E1s_{b}", tag=f"E1s_{b}") for b in range(2)]
            Ms = [eprep.tile([P, W], fp16, name=f"Ms_{b}", tag=f"Ms_{b}") for b in range(2)]
            aoff_t = [eprep.tile([P, J_OV], i32, name=f"aoff_{b}", tag=f"aoff_{b}") for b in range(2)]
            beta_rows = [eprep.tile([P, 1], f32, name=f"beta_{blk}",
                                    tag=f"beta_{blk}") for blk in range(2)]
            zoff_t = [eprep.tile([P, J_OV], i32, name=f"zoff_{b}", tag=f"zoff_{b}") for b in range(2)]
            for blk in range(2):
                rows_b = slice(blk * P, (blk + 1) * P)
                nc.sync.dma_start(out=aoff_t[blk][:], in_=d_aoff[rows_b, :])
                nc.sync.dma_start(out=zoff_t[blk][:], in_=d_zoff[rows_b, :])

            with tc.tile_pool(name="wts", bufs=1) as wpool:
                hcolT = wpool.tile([P, N], fp16, tag="hcolT")
                wsl = [wpool.tile([P, F], fp16, name=f"wsl_{i}", tag=f"wsl_{i}")
                       for i in range(3)]

                # =====================================================
                # Phase A: spectral part (column-sharded Chebyshev)
                # =====================================================
                with (
                    tc.tile_pool(name="adjp", bufs=1) as apool,
                    tc.tile_pool(name="awork", bufs=1) as aw,
                    tc.tile_pool(name="ps_set", bufs=1, space="PSUM") as ps_set,
                    tc.tile_pool(name="ps_a", bufs=2, space="PSUM") as ps_a,
                ):
                    _scA = nc.named_scope("phaseA"); _scA.__enter__()
                    adj_sb = [apool.tile([P, N], fp16, name=f"adj{t}", tag=f"adj{t}")
                              for t in range(NT)]
                    adj_qs = [nc.sync, nc.gpsimd, nc.scalar]
                    for t in range(NT):
                        adj_qs[t % 3].dma_start(out=adj_sb[t][:],
                                                in_=d_adj[ts(t, P), :])

                    rsc = aw.tile([P, N], fp16, tag="t_prev")  # pre-tau0 scratch
                    for t in range(NT):
                        if t % 2 == 0:
                            nc.vector.reduce_sum(dsum[:, t:t + 1], adj_sb[t][:],
                                                 axis=mybir.AxisListType.X)
                        else:
                            nc.scalar.activation(rsc[:], adj_sb[t][:],
                                                 AF.Copy,
                                                 accum_out=dsum[:, t:t + 1])
                    for q4 in range(4):
                        sl = slice(q4 * 4, (q4 + 1) * 4)
                        nc.vector.reciprocal(dinv2[:, sl], dsum[:, sl])
                        nc.scalar.activation(dinv[:, sl], dinv2[:, sl], AF.Sqrt)
                        nc.vector.tensor_tensor(out=sqd[:, sl],
                                                in0=dsum[:, sl],
                                                in1=dinv[:, sl], op=ALU.mult)
                        nc.vector.tensor_scalar(out=dinv2b[:, sl],
                                                in0=dinv2[:, sl],
                                                scalar1=2.0 / B_CHEB,
                                                scalar2=None, op0=ALU.mult)


                    def to_row(col_t, name):
                        ps_t = ps_set.tile([NT, P], f32, space="PSUM", tag="rowt")
                        nc.tensor.transpose(ps_t[:NT, :], col_t[:, :NT], id32[:])
                        sb_t = aw.tile([NT, P], f32, tag="rowt_sb", name="rowt_sb")
                        nc.vector.tensor_copy(sb_t[:NT, :], ps_t[:NT, :])
                        row = aw.tile([1, N], fp16, tag=f"row_{name}",
                                      name=f"row_{name}")
                        nc.gpsimd.dma_start(out=row[:1, :], in_=sb_t[:NT, :])
                        return row

                    d_rowv = to_row(dsum, "d")
                    sqd_row16 = to_row(sqd, "sqd")
                    z2 = aw.tile([1, 1], f32)
                    nc.vector.reduce_sum(z2[:1, :1], d_rowv[:1, :],
                                         axis=mybir.AxisListType.X)
                    rz2 = aw.tile([1, 1], f32)
                    nc.vector.reciprocal(rz2[:1, :1], z2[:1, :1])

                    # rank-1 scalars (1/Z2) are folded into uh_row / the css
                    # copies, so the row factors need only immediate scales.
                    negd_row = aw.tile([1, N], fp16, tag="negd")
                    nc.vector.tensor_scalar(out=negd_row[:], in0=d_rowv[:],
                                            scalar1=-1.0, scalar2=None,
                                            op0=ALU.mult)
                    negd2b_row = aw.tile([1, N], fp16, tag="negd2b")
                    nc.vector.tensor_scalar(out=negd2b_row[:], in0=d_rowv[:],
                                            scalar1=-2.0 / B_CHEB, scalar2=None,
                                            op0=ALU.mult)
                    sqd_row_e4 = aw.tile([1, N], fp16, tag="sqde4")
                    nc.vector.tensor_scalar(out=sqd_row_e4[:], in0=sqd_row16[:],
                                            scalar1=float(np.exp(-4.0)),
                                            scalar2=None, op0=ALU.mult)

                    t_prev = aw.tile([P, N], fp16, tag="t_prev")
                    t_cur = aw.tile([P, N], fp16, tag="t_cur")
                    tn_tmp = aw.tile([P, N], fp16, tag="tn_tmp")
                    v_sc = aw.tile([P, N], fp16, tag="v_sc")
                    y1t = aw.tile([P, N], fp16, tag="y1t")
                    y2t = aw.tile([P, N], fp16, tag="y2t")
                    css = aw.tile([1, P], fp16, tag="css")

                    # h column slice staged through tn_tmp (reused later)
                    for t in range(NT):
                        nc.sync.dma_start(out=tn_tmp[:, ts(t, P)],
                                            in_=d_hcol[ts(t, P), :])
                    hs = aw.tile([P, N], fp16, tag="hs")
                    for t in range(NT):
                        nc.scalar.activation(hs[:, ts(t, P)], tn_tmp[:, ts(t, P)],
                                             AF.Copy, scale=sqd[:, t:t + 1])

                    ps_cs = ps_set.tile([1, P], f32, space="PSUM", tag="cs")
                    for t in range(NT):
                        nc.tensor.matmul(ps_cs[:1, :], ones_c16[:, :1],
                                         hs[:, ts(t, P)],
                                         start=(t == 0), stop=(t == NT - 1))
                    p0_row = aw.tile([1, P], f32, tag="p0")
                    nc.vector.tensor_copy(p0_row[:1, :], ps_cs[:1, :])
                    uh_row = aw.tile([1, P], fp16, tag="uh")
                    nc.vector.tensor_scalar(out=uh_row[:1, :], in0=p0_row[:1, :],
                                            scalar1=rz2[:1, :1], scalar2=None,
                                            op0=ALU.mult)
                    p0_row16 = aw.tile([1, P], fp16, tag="p016")
                    nc.vector.tensor_copy(p0_row16[:1, :], p0_row[:1, :])

                    # Software-pipelined recurrence (v/css ping-pong buffers)
                    v_nx = hs  # alias: hs is dead after tau0; reuse as 2nd v buf
                    css2 = aw.tile([1, P], fp16, tag="css2")
                    vbuf = [v_sc, v_nx]
                    csbuf = [css, css2]

                    def tail_update(dst_t, m, k):
                        """after t_{k}[m] lands: v-scale for k+1."""
                        if k == DEG:
                            return
                        nc.scalar.activation(vbuf[(k + 1) % 2][:, ts(m, P)],
                                             dst_t[:, ts(m, P)], AF.Copy,
                                             scale=dinv2b[:, m:m + 1])

                    def colsum_batch(dst_t, k):
                        """contiguous colsum group of T_k -> csbuf[(k+1)%2]."""
                        if k == DEG:
                            return
                        ps_c = ps_set.tile([1, P], f32, space="PSUM",
                                           tag="csA", name=f"ps_cs_{k}")
                        for m in range(NT):
                            nc.tensor.matmul(ps_c[:1, :], ones_c16[:, :1],
                                             dst_t[:, ts(m, P)], start=(m == 0),
                                             stop=(m == NT - 1))
                        nc.scalar.activation(csbuf[(k + 1) % 2][:1, :],
                                             ps_c[:1, :], AF.Copy,
                                             scale=rz2[:1, :1])

                    # tau0 = hs - d (1^T hs)/Z2   (k=0 stage of the pipeline)
                    for m in range(NT):
                        ps_m = ps_a.tile([P, P], f32, space="PSUM", tag="psm")
                        nc.tensor.matmul(ps_m[:], negd_row[:1, ts(m, P)],
                                         uh_row[:1, :], start=True, stop=True)
                        nc.vector.tensor_tensor(out=t_prev[:, ts(m, P)],
                                                in0=hs[:, ts(m, P)], in1=ps_m[:],
                                                op=ALU.add)
                        tail_update(t_prev, m, 0)
                    colsum_batch(t_prev, 0)
                    nc.vector.tensor_scalar(out=y1t[:], in0=t_prev[:],
                                            scalar1=float(cg[0]), scalar2=None,
                                            op0=ALU.mult)
                    nc.vector.tensor_scalar(out=y2t[:], in0=t_prev[:],
                                            scalar1=float(cf[0]), scalar2=None,
                                            op0=ALU.mult)
                    # hcolT / W-slice loads (adj DMAs have priority at start)
                    nc.gpsimd.dma_start(out=hcolT[:], in_=d_hcolT[:, :])
                    for i in range(3):
                        nc.gpsimd.dma_start(out=wsl[i][:],
                                            in_=d_wsl[ts(i, P), :])

                    # ---- edge-phase scatter prep (gpsimd idle during cheb) ----
                    for blk in range(2):
                        rows_b = slice(blk * P, (blk + 1) * P)
                        for hf in (0, 1):
                            idx_t = eprep.tile([P, J0], i16, name=f"idx_{blk}_{hf}", tag=f"idx_{blk}_{hf}")
                            nc.sync.dma_start(out=idx_t[:], in_=d_idx0[hf][rows_b, :])
                            e0_t = eprep.tile([P, J0], fp16, name=f"e0t_{blk}_{hf}", tag=f"e0t_{blk}_{hf}")
                            nc.sync.dma_start(out=e0_t[:], in_=d_e0h[hf][rows_b, :])
                            e1_t = eprep.tile([P, J0], fp16, name=f"e1t_{blk}_{hf}", tag=f"e1t_{blk}_{hf}")
                            nc.sync.dma_start(out=e1_t[:], in_=d_e1h[hf][rows_b, :])
                            nc.gpsimd.local_scatter(
                                E0s[blk][:, hf * 1024:(hf + 1) * 1024],
                                e0_t[:], idx_t[:], channels=P,
                                num_elems=1024, num_idxs=J0)
                            nc.gpsimd.local_scatter(
                                E1s[blk][:, hf * 1024:(hf + 1) * 1024],
                                e1_t[:], idx_t[:], channels=P,
                                num_elems=1024, num_idxs=J0)
                            nc.gpsimd.local_scatter(
                                Ms[blk][:, hf * 1024:(hf + 1) * 1024],
                                ones_scat[:], idx_t[:], channels=P,
                                num_elems=1024, num_idxs=J0)
                        nc.sync.dma_start(out=E0s[blk][:, N:N + J_OV],
                                          in_=d_e0o[rows_b, :])
                        nc.sync.dma_start(out=E1s[blk][:, N:N + J_OV],
                                          in_=d_e1o[rows_b, :])
                        nc.sync.dma_start(out=Ms[blk][:, N:N + J_OV],
                                          in_=d_mo[rows_b, :])
                        if W > N + J_OV:
                            nc.vector.memset(E0s[blk][:, N + J_OV:], 0.0)
                            nc.vector.memset(E1s[blk][:, N + J_OV:], 0.0)
                            nc.vector.memset(Ms[blk][:, N + J_OV:], 0.0)

                    # ---- Chebyshev recurrence ----
                    for k in range(1, DEG + 1):
                        vcur = vbuf[k % 2]
                        ccur = csbuf[k % 2]
                        dst_t = t_cur if k == 1 else t_prev
                        for m in range(NT):
                            ps_m = ps_a.tile([P, P], f32, space="PSUM", tag="psm")
                            for kk in range(NT):
                                nc.tensor.matmul(ps_m[:], adj_sb[kk][:, ts(m, P)],
                                                 vcur[:, ts(kk, P)],
                                                 start=(kk == 0), stop=False)
                            nc.tensor.matmul(ps_m[:], negd2b_row[:1, ts(m, P)],
                                             ccur[:1, :], start=False, stop=True)
                            if k == 1:
                                nc.vector.tensor_scalar(
                                    out=dst_t[:, ts(m, P)], in0=ps_m[:],
                                    scalar1=0.5, scalar2=None, op0=ALU.mult)
                            else:
                                nc.vector.scalar_tensor_tensor(
                                    out=dst_t[:, ts(m, P)], in0=ps_m[:],
                                    scalar=1.0, in1=dst_t[:, ts(m, P)],
                                    op0=ALU.mult, op1=ALU.subtract)
                            tail_update(dst_t, m, k)
                        colsum_batch(dst_t, k)
                        if k > 1:
                            t_prev, t_cur = t_cur, t_prev
                        tgt = t_cur
                        if abs(cg[k]) > 1e-7:
                            nc.vector.scalar_tensor_tensor(
                                out=y1t[:], in0=tgt[:], scalar=float(cg[k]),
                                in1=y1t[:], op0=ALU.mult, op1=ALU.add)
                        if abs(cf[k]) > 1e-7:
                            nc.vector.scalar_tensor_tensor(
                                out=y2t[:], in0=tgt[:], scalar=float(cf[k]),
                                in1=y2t[:], op0=ALU.mult, op1=ALU.add)

                    # wa[:, i] = W_i[my cols, :] @ a1 ; biasa8 = (bias @ a1)/8
                    watmp = aw.tile([P, F], fp16, tag="watmp")
                    wa32 = aw.tile([P, 4], f32, tag="wa32")
                    for i in range(3):
                        nc.vector.tensor_tensor(out=watmp[:], in0=wsl[i][:],
                                                in1=a1B[:], op=ALU.mult)
                        nc.vector.reduce_sum(wa32[:, i:i + 1], watmp[:],
                                             axis=mybir.AxisListType.X)
                    nc.vector.tensor_copy(wa[:, :3], wa32[:, :3])
                    batmp = aw.tile([1, F], fp16, tag="batmp")
                    nc.vector.tensor_tensor(out=batmp[:1, :], in0=bias16[:1, :],
                                            in1=a1_16[:1, :], op=ALU.mult)
                    bsum = aw.tile([1, 2], f32, tag="bsum")
                    nc.vector.reduce_sum(bsum[:1, 0:1], batmp[:1, :],
                                         axis=mybir.AxisListType.X)
                    nc.vector.tensor_scalar(out=biasa8[:1, 0:1],
                                            in0=bsum[:1, 0:1],
                                            scalar1=1.0 / C, scalar2=None,
                                            op0=ALU.mult)
                    # y_i = D^-1/2 y_i~ + addback*sqrt(d)(u^T h); write y^T
                    # blocks straight into the A2A layout (no phase-B
                    # transposes), and accumulate the alpha partials
                    # alpha_c = h_c@wa1 + y1_c@wa2 + y2_c@wa3 + bias.a1/8
                    y16 = v_sc
                    ytmp = tn_tmp  # tn_tmp is dead after the h staging
                    ywide = [[aw.tile([P, R], fp16, name=f"yw_{h}_{b}",
                                      tag=f"yw_{h}_{b}") for b in range(2)]
                             for h in range(2)]
                    acol = aw.tile([P, NT], f32, tag="acol")
                    for (yt, lrow, half) in ((y1t, sqd_row16, 0),
                                             (y2t, sqd_row_e4, 1)):
                        for m in range(NT):
                            ps_m = ps_a.tile([P, P], f32, space="PSUM", tag="psm")
                            nc.tensor.matmul(ps_m[:], lrow[:1, ts(m, P)],
                                             uh_row[:1, :], start=True, stop=True)
                            nc.scalar.activation(ytmp[:, ts(m, P)],
                                                 yt[:, ts(m, P)], AF.Copy,
                                                 scale=dinv[:, m:m + 1])
                            nc.vector.tensor_tensor(out=y16[:, ts(m, P)],
                                                    in0=ytmp[:, ts(m, P)],
                                                    in1=ps_m[:], op=ALU.add)
                            ps_yt = ps_set.tile([P, P], fp16, space="PSUM",
                                                tag=("ytp" if m % 2 == 0
                                                     else "ytp2"),
                                                name=f"yt_{m}_{half}")
                            nc.tensor.transpose(ps_yt[:], y16[:, ts(m, P)],
                                                ident[:])
                            yw = ywide[half][(m // 2) % 2]
                            nc.vector.tensor_copy(
                                yw[:, (m % 2) * P:(m % 2 + 1) * P], ps_yt[:])
                            if m % 2 == 1:
                                r0 = (m // 2) * R + half * P
                                nc.sync.dma_start(out=y12T[r0:r0 + P, :],
                                                  in_=yw[:])
                            ps_ab = ps_set.tile([P, 1], f32, space="PSUM",
                                                tag="pab", name=f"pab_{m}_{half}")
                            ybs = yw[:, (m % 2) * P:(m % 2 + 1) * P]
                            if half == 0:
                                nc.tensor.matmul(ps_ab[:, :1], hcolT[:, ts(m, P)],
                                                 wa[:, 0:1], start=True,
                                                 stop=False)
                                nc.tensor.matmul(ps_ab[:, :1], ybs, wa[:, 1:2],
                                                 start=False, stop=False)
                                nc.tensor.matmul(ps_ab[:, :1], ones_r16[:1, :],
                                                 biasa8[:1, 0:1], start=False,
                                                 stop=True)
                                nc.vector.tensor_copy(acol[:, m:m + 1],
                                                      ps_ab[:, :1])
                            else:
                                nc.tensor.matmul(ps_ab[:, :1], ybs, wa[:, 2:3],
                                                 start=True, stop=True)
                                nc.vector.tensor_tensor(out=acol[:, m:m + 1],
                                                        in0=acol[:, m:m + 1],
                                                        in1=ps_ab[:, :1],
                                                        op=ALU.add)
                    # acol [P, NT] -> node-flat [N, 1] via transpose + one DMA
                    ps_at = ps_set.tile([NT, P], f32, space="PSUM", tag="rowt",
                                        name="ps_at")
                    nc.tensor.transpose(ps_at[:NT, :], acol[:, :NT], id32[:])
                    acolT = aw.tile([NT, P], f32, tag="acolT")
                    nc.vector.tensor_copy(acolT[:NT, :], ps_at[:NT, :])
                    nc.sync.dma_start(out=ab_part[0:N, 0:1], in_=acolT[:NT, :])

                    _scA.__exit__(None, None, None)
                    _scC1 = nc.named_scope("a2a"); _scC1.__enter__()
                    with tc.high_priority():
                        nc.gpsimd.collective_compute(
                            "AllToAll", ALU.bypass, ins=[y12T[:]],
                            outs=[y12x[:]], replica_groups=rgroups)
                        nc.gpsimd.collective_compute(
                            "AllReduce", ALU.add, ins=[ab_part[:]],
                            outs=[abg[:]], replica_groups=rgroups)
                    _scC1.__exit__(None, None, None)

                # =====================================================
                # Phase B: z rows = h@W1 + y1@W2 + y2@W3 + bias
                # (pure matmuls; h-part runs during the A2A)
                # =====================================================
                with (
                    tc.tile_pool(name="bwork", bufs=1) as bw,
                    tc.tile_pool(name="ps_b", bufs=1, space="PSUM") as ps_b,
                ):
                    _scB = nc.named_scope("phaseB"); _scB.__enter__()
                    w_sb = [[bw.tile([P, F], fp16, name=f"w{i}_{k}",
                             tag=f"w{i}_{k}") for k in range(KT)]
                            for i in range(3)]
                    hT_sb = [bw.tile([P, R], fp16, name=f"hT_{k}",
                             tag=f"hT_{k}") for k in range(KT)]
                    for k in range(KT):
                        nc.gpsimd.dma_start(out=w_sb[0][k][:],
                                            in_=d_w[0][ts(k, P), :])
                    for k in range(KT):
                        nc.gpsimd.dma_start(out=hT_sb[k][:],
                                            in_=d_hrowT[ts(k, P), :])
                    for i in (1, 2):
                        for k in range(KT):
                            nc.gpsimd.dma_start(out=w_sb[i][k][:],
                                                in_=d_w[i][ts(k, P), :])
                    ps_h = [[ps_b.tile([P, 512], f32, space="PSUM",
                                       name=f"h{blk}{chunk}", tag=f"h{blk}{chunk}")
                              for chunk in range(2)] for blk in range(2)]
                    ps_y = [[ps_b.tile([P, 512], f32, space="PSUM",
                                       name=f"y{blk}{chunk}", tag=f"y{blk}{chunk}")
                              for chunk in range(2)] for blk in range(2)]
                    # bias + h@W1: contiguous groups, independent of the A2As
                    for blk in range(2):
                        for chunk in range(2):
                            ph = ps_h[blk][chunk]
                            nc.tensor.matmul(ph[:], ones_r16[:1, :],
                                             bias16[:1, ts(chunk, 512)],
                                             start=True, stop=False)
                            for k in range(KT):
                                nc.tensor.matmul(
                                    ph[:], hT_sb[k][:, ts(blk, P)],
                                    w_sb[0][k][:, ts(chunk, 512)],
                                    start=False, stop=(k == KT - 1))
                    # y part: per-bank contiguous groups (y1 then y2)
                    yx = [[bw.tile([P, R], fp16, name=f"yx_{yi}_{s}",
                                   tag=f"yx_{yi}_{s}") for s in range(C)]
                          for yi in range(2)]
                    for yi in range(2):
                        for s in range(C):
                            q = nc.sync if s % 2 == 0 else nc.gpsimd
                            q.dma_start(
                                out=yx[yi][s][:],
                                in_=y12x[s * R + yi * P:s * R + (yi + 1) * P, :])
                    for blk in range(2):
                        for chunk in range(2):
                            py = ps_y[blk][chunk]
                            for yi in range(2):
                                for s in range(C):
                                    nc.tensor.matmul(
                                        py[:], yx[yi][s][:, ts(blk, P)],
                                        w_sb[1 + yi][s][:, ts(chunk, 512)],
                                        start=(yi == 0 and s == 0),
                                        stop=(yi == 1 and s == C - 1))
                    for blk in range(2):
                        z16 = bw.tile([P, F], fp16, tag=f"z16_{blk}",
                                      name=f"z16_{blk}")
                        for chunk in range(2):
                            nc.scalar.activation(z16[:, ts(chunk, 512)],
                                                 ps_h[blk][chunk][:], AF.Copy)
                            nc.vector.scalar_tensor_tensor(
                                out=z16[:, ts(chunk, 512)],
                                in0=ps_y[blk][chunk][:], scalar=1.0,
                                in1=z16[:, ts(chunk, 512)],
                                op0=ALU.mult, op1=ALU.add)
                        nc.sync.dma_start(out=z_slice[ts(blk, P), :], in_=z16[:])
                        abtmp = bw.tile([P, F], fp16, tag=f"abtmp_{blk}",
                                        name=f"abtmp_{blk}")
                        nc.vector.tensor_tensor(out=abtmp[:], in0=z16[:],
                                                in1=a2B[:], op=ALU.mult)
                        nc.vector.reduce_sum(beta_rows[blk][:, 0:1], abtmp[:],
                                             axis=mybir.AxisListType.X)

                    _scB.__exit__(None, None, None)
                    _scC2 = nc.named_scope("ags"); _scC2.__enter__()
                    with tc.high_priority():
                        nc.gpsimd.collective_compute(
                            "AllGather", ALU.bypass, ins=[z_slice[:]],
                            outs=[zg[:]], replica_groups=rgroups)
                    _scC2.__exit__(None, None, None)

            # =========================================================
            # Edge phase (row-sharded dense layered softmax).
            # Softmax chain overlaps the z AllGather; z-dependent work last.
            # =========================================================
            with (
                tc.tile_pool(name="edge", bufs=1) as ep,
                tc.tile_pool(name="edge2", bufs=2) as ep2,
                tc.tile_pool(name="ps_e", bufs=2, space="PSUM") as ps_e,
            ):
                _scE = nc.named_scope("edge"); _scE.__enter__()
                # sync queue: al_row (waits ab-AG) BEFORE z_sb (waits z-AG)
                al_row = ep.tile([1, N], f32, tag="al_row")
                nc.sync.dma_start(out=al_row[:1, :], in_=abg[0:N, :1])
                # alo gathers (need only the alpha AllReduce)
                alo = [ep.tile([P, J_OV], f32, name=f"alo_{b}", tag=f"alo_{b}") for b in range(2)]
                for blk in range(2):
                    for j in range(J_OV):
                        nc.gpsimd.indirect_dma_start(
                            out=alo[blk][:, j:j + 1], out_offset=None,
                            in_=abg[:],
                            in_offset=bass.IndirectOffsetOnAxis(
                                ap=aoff_t[blk][:, j:j + 1], axis=0))
                # z tiles split across both DMA queues; zo gathers last
                z_sb = [ep.tile([P, F], fp16, name=f"z_{t}", tag=f"z_{t}")
                        for t in range(NT)]
                z_qs = [nc.sync, nc.gpsimd, nc.scalar]
                for t in range(NT):
                    z_qs[t % 3].dma_start(out=z_sb[t][:], in_=zg[ts(t, P), :])
                zo_t = [[ep.tile([P, F], fp16, name=f"zo_{blk}_{j}",
                                 tag=f"zo_{blk}_{j}") for j in range(J_OV)]
                        for blk in range(2)]
                for blk in range(2):
                    for j in range(J_OV):
                        nc.gpsimd.indirect_dma_start(
                            out=zo_t[blk][j][:], out_offset=None, in_=zg[:],
                            in_offset=bass.IndirectOffsetOnAxis(
                                ap=zoff_t[blk][:, j:j + 1], axis=0))

                # vector: xp (no deps — runs during the AGs)
                xp = [ep.tile([P, W], fp16, name=f"xp_{b}", tag=f"xp_{b}") for b in range(2)]
                for blk in range(2):
                    nc.vector.tensor_scalar(out=xp[blk][:], in0=E1s[blk][:],
                                            scalar1=v01b[:, 1:2], scalar2=None,
                                            op0=ALU.mult)
                    nc.vector.scalar_tensor_tensor(out=xp[blk][:],
                                                   in0=E0s[blk][:],
                                                   scalar=v01b[:, 0:1],
                                                   in1=xp[blk][:],
                                                   op0=ALU.mult, op1=ALU.add)

                # alB broadcast (needs ab-AG)
                al_row16 = ep.tile([1, N], fp16, tag="al_row16")
                nc.vector.tensor_copy(al_row16[:1, :], al_row[:1, :])
                alB = ep.tile([P, N], fp16, tag="alB")
                for chunk in range(N // 512):
                    ps_bb = ps_e.tile([P, 512], f32, space="PSUM", tag="bc")
                    nc.tensor.matmul(ps_bb[:], ones_r16[:1, :],
                                     al_row16[:1, ts(chunk, 512)],
                                     start=True, stop=True)
                    nc.scalar.activation(alB[:, ts(chunk, 512)], ps_bb[:],
                                         AF.Copy)

                # loop 1: softmax build per blk (no z dependence)
                pmat = [ep.tile([P, W], fp16, name=f"pmat_{b}", tag=f"pmat_{b}") for b in range(2)]
                denom = [ep.tile([P, 1], f32, name=f"denom_{b}", tag=f"denom_{b}") for b in range(2)]
                qqT = [ep.tile([2, P], fp16, name=f"qqT_{b}", tag=f"qqT_{b}") for b in range(2)]
                PT = [ep.tile([P, N], fp16, name=f"PT_{b}", tag=f"PT_{b}") for b in range(2)]
                for blk in range(2):
                    beta_blk = beta_rows[blk][:, 0:1]
                    alo_b = ep2.tile([P, J_OV], f32, tag="alo_b")
                    nc.vector.tensor_scalar(out=alo_b[:], in0=alo[blk][:],
                                            scalar1=beta_blk, scalar2=None,
                                            op0=ALU.add)
                    x1 = ep2.tile([P, W], fp16, tag="x1")
                    nc.vector.scalar_tensor_tensor(out=x1[:, 0:N],
                                                   in0=xp[blk][:, 0:N],
                                                   scalar=beta_blk, in1=alB[:],
                                                   op0=ALU.add, op1=ALU.add)
                    nc.vector.tensor_copy(x1[:, N:W], xp[blk][:, N:W])
                    nc.vector.tensor_tensor(out=x1[:, N:N + J_OV],
                                            in0=xp[blk][:, N:N + J_OV],
                                            in1=alo_b[:], op=ALU.add)
                    nc.vector.scalar_tensor_tensor(out=x1[:], in0=x1[:],
                                                   scalar=0.01, in1=x1[:],
                                                   op0=ALU.mult, op1=ALU.max)
                    xm = ep2.tile([P, W], f32, tag="xm")
                    nc.vector.scalar_tensor_tensor(out=xm[:], in0=Ms[blk][:],
                                                   scalar=BIG, in1=x1[:],
                                                   op0=ALU.mult, op1=ALU.add)
                    mx = ep2.tile([P, 1], f32, tag="mx")
                    nc.vector.reduce_max(mx[:], xm[:],
                                         axis=mybir.AxisListType.X)
                    negmx = ep2.tile([P, 1], f32, tag="negmx")
                    nc.vector.tensor_scalar(out=negmx[:], in0=mx[:],
                                            scalar1=-1.0, scalar2=None,
                                            op0=ALU.mult)
                    nc.scalar.activation(pmat[blk][:], xm[:], AF.Exp,
                                         bias=negmx[:, :1],
                                         accum_out=denom[blk][:, :1])
                    s01 = ep2.tile([P, 2], f32, tag="s01")
                    x2 = ep2.tile([P, W], fp16, tag="x2")
                    for (j, Es) in ((0, E0s[blk]), (1, E1s[blk])):
                        nc.vector.scalar_tensor_tensor(
                            out=x2[:], in0=pmat[blk][:], scalar=1.0, in1=Es[:],
                            op0=ALU.mult, op1=ALU.mult,
                            accum_out=s01[:, j:j + 1])
                    q01 = ep2.tile([P, 2], fp16, tag="q01")
                    qtmp = ep2.tile([P, 1], f32, tag="qtmp")
                    for (j, ca, cb) in ((0, ewb[:, 0:1], ewb[:, 1:2]),
                                        (1, ewb[:, 2:3], ewb[:, 3:4])):
                        nc.vector.tensor_scalar(out=qtmp[:], in0=s01[:, 0:1],
                                                scalar1=ca[:, :1], scalar2=None,
                                                op0=ALU.mult)
                        nc.vector.scalar_tensor_tensor(out=q01[:, j:j + 1],
                                                       in0=s01[:, 1:2],
                                                       scalar=cb[:, :1],
                                                       in1=qtmp[:],
                                                       op0=ALU.mult, op1=ALU.add)
                    ps_q = ps_e.tile([P, P], fp16, space="PSUM", tag="tp")
                    nc.tensor.transpose(ps_q[:2, :], q01[:], ident[:])
                    nc.vector.tensor_copy(qqT[blk][:2, :], ps_q[:2, :])
                    for t in range(NT):
                        ps_t = ps_e.tile([P, P], fp16, space="PSUM", tag="tp")
                        nc.tensor.transpose(ps_t[:], pmat[blk][:, ts(t, P)],
                                            ident[:])
                        nc.vector.tensor_copy(PT[blk][:, ts(t, P)], ps_t[:])

                # loop 2: z-dependent matmuls + overflow + output
                for blk in range(2):
                    rows = slice(blk * P, (blk + 1) * P)
                    out_sb = ep2.tile([P, F], fp16, tag="out_sb")
                    for chunk in range(2):
                        ps_o = ps_e.tile([P, 512], f32, space="PSUM", tag="pso")
                        nc.tensor.matmul(ps_o[:], qqT[blk][:2, :],
                                         e2nT[:2, ts(chunk, 512)],
                                         start=True, stop=False)
                        for t in range(NT):
                            nc.tensor.matmul(ps_o[:], PT[blk][:, ts(t, P)],
                                             z_sb[t][:, ts(chunk, 512)],
                                             start=False, stop=(t == NT - 1))
                        nc.vector.tensor_copy(out_sb[:, ts(chunk, 512)],
                                              ps_o[:])

                    po32 = ep2.tile([P, J_OV], f32, tag="po32")
                    nc.vector.tensor_copy(po32[:], pmat[blk][:, N:N + J_OV])
                    for j in range(J_OV):
                        nc.vector.scalar_tensor_tensor(
                            out=out_sb[:], in0=zo_t[blk][j][:],
                            scalar=po32[:, j:j + 1], in1=out_sb[:],
                            op0=ALU.mult, op1=ALU.add)

                    recipd = ep2.tile([P, 1], f32, tag="recipd")
                    nc.vector.reciprocal(recipd[:], denom[blk][:])
                    out_f = ep2.tile([P, F], f32, tag="out_f")
                    nc.scalar.activation(out_f[:], out_sb[:], AF.Copy,
                                         scale=recipd[:, :1])
                    nc.sync.dma_start(out=d_out[rows, :], in_=out_f[:])
                _scE.__exit__(None, None, None)

    nc.compile()
    return nc


_PROGRAM_CACHE = {}


def kernel(**inputs):
    h = np.asarray(inputs["h"], np.float32)
    e = np.asarray(inputs["e"], np.float32)
    adj = np.asarray(inputs["adj"], np.float32)
    src = np.asarray(inputs["src"])
    dst = np.asarray(inputs["dst"])
    weight = np.asarray(inputs["weight"], np.float32)
    weight2 = np.asarray(inputs["weight2"], np.float32)
    weight3 = np.asarray(inputs["weight3"], np.float32)
    bias = np.asarray(inputs["bias"], np.float32)
    attn_w = np.asarray(inputs["attn_w"], np.float32)
    edge_w = np.asarray(inputs["edge_w"], np.float32)
    e2n_w = np.asarray(inputs["e2n_w"], np.float32)

    halves, J0, ov, J_OV = _host_prep(e, src, dst)
    e0o, e1o, mo, aoff, zoff = ov

    key = (J0, J_OV)
    if key not in _PROGRAM_CACHE:
        _PROGRAM_CACHE[key] = _build_program(J0, J_OV)
    nc = _PROGRAM_CACHE[key]

    adj16 = adj.astype(np.float16)
    h16 = h.astype(np.float16)
    w16 = [weight[0].astype(np.float16), weight2[0].astype(np.float16),
           weight3[0].astype(np.float16)]

    in_maps = []
    for c in range(C):
        rows = slice(c * R, (c + 1) * R)
        m = {
            "adj": adj16,
            "hcol": np.ascontiguousarray(h16[:, c * COLS:(c + 1) * COLS]),
            "hrowT": np.ascontiguousarray(h16[rows, :].T),
            "hcolT": np.ascontiguousarray(h16[:, c * COLS:(c + 1) * COLS].T),
            "wsl": np.ascontiguousarray(np.concatenate(
                [w16[i][c * COLS:(c + 1) * COLS, :] for i in range(3)])),
            "w1": w16[0], "w2": w16[1], "w3": w16[2],
            "biasv": bias.reshape(1, F),
            "attnw": attn_w.reshape(1, 2 * F + 2),
            "edgew": edge_w,
            "e2nw": e2n_w,
            "e0o": np.ascontiguousarray(e0o[rows]).astype(np.float16),
            "e1o": np.ascontiguousarray(e1o[rows]).astype(np.float16),
            "mo": np.ascontiguousarray(mo[rows]).astype(np.float16),
            "aoff": np.ascontiguousarray(aoff[rows]),
            "zoff": np.ascontiguousarray(zoff[rows]),
        }
        for hf in (0, 1):
            idx_arr, e0_arr, e1_arr = halves[hf]
            m[f"idx0{hf}"] = np.ascontiguousarray(idx_arr[rows])
            m[f"e0h{hf}"] = np.ascontiguousarray(e0_arr[rows]).astype(np.float16)
            m[f"e1h{hf}"] = np.ascontiguousarray(e1_arr[rows]).astype(np.float16)
        in_maps.append(m)

    import os
    trace = bool(os.environ.get("BASS_GNN_TRACE"))
    res = run_bass_kernel_spmd(nc, in_maps, core_ids=list(range(C)),
                               trace=trace)
    if trace:
        kernel.last_results = res
    out = np.empty((N, F), np.float32)
    for c in range(C):
        out[c * R:(c + 1) * R] = res.results[c]["out_rows"]
    return out


if __name__ == "__main__":
    D = np.load("/tmp/refdata.npz")
    inp = {k: D[k] for k in D.files if k != "expected"}
    out = kernel(**inp)
    exp = D["expected"]
    rel = np.linalg.norm(out - exp) / np.linalg.norm(exp)
    print("rel err:", rel)


# revision 42
# speedup vs baseline: 1.0054x; 1.0054x over previous
"""Trainium2 Bass kernel for nn_BlockLayer_75376676045426 (gnn_message_passing).

Math (N=2048 nodes, E=67584 edges, F=1024 features, 8 NeuronCores):
  L = I - D^-1/2 A D^-1/2,  S = D^-1/2 A D^-1/2.  The reference's
  eigh-based wavelet weights are analytic functions of S:
      w1 = exp(-2L) = g(S),   w2 = exp(-4 exp(-2L)) = f(S).
  S has the Perron pair (lambda=1, u = sqrt(d)/||sqrt(d)||) in closed form;
  after deflating it exactly, the rest of the spectrum sits inside
  [-0.4, 0.4], so w1@h, w2@h are evaluated with a single shared degree-3
  Chebyshev recurrence.
  r = h@W1 + (w1 h)@W2 + (w2 h)@W3 + bias;  then GAT-style edge softmax:
  logits_e = alpha[src] + beta[dst] + gamma_e; segment softmax over dst;
  out = P@z + rank-2 term, with the dense attention matrix P built on-chip
  via gpsimd local_scatter (multi-edge duplicates go to overflow columns).

Sharding: phase A column-parallel (adj replicated in SBUF fp16, h columns
split 8 ways); the A2A payload is written pre-transposed so phase B is
pure matmuls; phase B h@W1 part runs during the A2A; edge-phase scatter
runs during the Chebyshev recurrence; softmax overlaps the z AllGather.
"""

import sys

sys.path.insert(0, "/opt/trn_rl_repo")

import numpy as np
from numpy.polynomial import chebyshev as _cheb

import concourse.bacc as bacc
import concourse.bass as bass
import concourse.mybir as mybir
import concourse.tile as tile
from concourse.bass_utils import run_bass_kernel_spmd
from concourse.masks import make_identity

P = 128
N = 2048
F = 1024
C = 8            # cores
R = N // C       # dst rows per core (256)
NT = N // P      # 16 node tiles
KT = F // P      # 8 feature tiles
COLS = F // C    # 128 h-columns per core
B_CHEB = 0.37    # Chebyshev half-width for the bulk spectrum of S
DEG = 2
BIG = 30000.0

fp16 = mybir.dt.float16
f32 = mybir.dt.float32
i16 = mybir.dt.int16
i32 = mybir.dt.int32
AF = mybir.ActivationFunctionType
ALU = mybir.AluOpType
ts = bass.ts


def _cheb_coeffs():
    g = lambda y: np.exp(-2.0 * (1.0 - B_CHEB * y))
    f = lambda y: np.exp(-4.0 * np.exp(-2.0 * (1.0 - B_CHEB * y)))
    return (_cheb.chebinterpolate(g, DEG).astype(np.float64),
            _cheb.chebinterpolate(f, DEG).astype(np.float64))


def _host_prep(e, src, dst):
    """Index/layout-only host prep: stable sort by (dst, src), padded
    per-row scatter layouts, overflow slots for duplicate (dst, src) cells."""
    src = np.asarray(src).astype(np.int64)
    dst = np.asarray(dst).astype(np.int64)
    e = np.asarray(e)
    E = src.shape[0]
    order = np.lexsort((src, dst))
    ds, ss = dst[order], src[order]
    eo = np.ascontiguousarray(e[order])

    cell = ds * N + ss
    first = np.r_[True, cell[1:] != cell[:-1]]
    idxs = np.arange(E)
    ranks = idxs - np.maximum.accumulate(np.where(first, idxs, 0))

    l0 = ranks == 0
    J0 = 0
    for hf in (0, 1):
        sel = l0 & ((ss // 1024) == hf)
        J0 = max(J0, int(np.bincount(ds[sel], minlength=N).max()))
    J0 = (J0 + 1) // 2 * 2
    halves = []
    for hf in (0, 1):
        sel = np.where(l0 & ((ss // 1024) == hf))[0]
        idx_arr = np.full((N, J0), -1, np.int16)
        e0_arr = np.zeros((N, J0), np.float32)
        e1_arr = np.zeros((N, J0), np.float32)
        pos = np.zeros(N, np.int64)
        for k in sel:
            n = ds[k]
            j = pos[n]; pos[n] = j + 1
            idx_arr[n, j] = ss[k] - 1024 * hf
            e0_arr[n, j] = eo[k, 0]
            e1_arr[n, j] = eo[k, 1]
        halves.append((idx_arr, e0_arr, e1_arr))

    ov = np.where(ranks >= 1)[0]
    J_OV = max(2, int(np.bincount(ds[ov], minlength=N).max()) if len(ov) else 2)
    J_OV = (J_OV + 1) // 2 * 2
    e0o = np.zeros((N, J_OV), np.float32)
    e1o = np.zeros((N, J_OV), np.float32)
    mo = np.zeros((N, J_OV), np.float32)
    aoff = np.zeros((N, J_OV), np.int32)
    zoff = np.zeros((N, J_OV), np.int32)
    pos = np.zeros(N, np.int64)
    for k in ov:
        n = ds[k]
        j = pos[n]; pos[n] = j + 1
        e0o[n, j] = eo[k, 0]
        e1o[n, j] = eo[k, 1]
        mo[n, j] = 1.0
        s = int(ss[k])
        aoff[n, j] = s
        zoff[n, j] = s
    return halves, J0, (e0o, e1o, mo, aoff, zoff), J_OV


def _build_program(J0, J_OV):
    cg, cf = _cheb_coeffs()
    W = N + ((J_OV + 7) // 8) * 8
    nc = bacc.Bacc("TRN2", target_bir_lowering=False, debug=False, num_devices=C)

    # ---------------- DRAM I/O ----------------
    d_adj = nc.dram_tensor("adj", [N, N], fp16, kind="ExternalInput").ap()
    d_hcol = nc.dram_tensor("hcol", [N, COLS], fp16, kind="ExternalInput").ap()
    d_hcolT = nc.dram_tensor("hcolT", [COLS, N], fp16, kind="ExternalInput").ap()
    d_wsl = nc.dram_tensor("wsl", [3 * P, F], fp16, kind="ExternalInput").ap()
    d_hrowT = nc.dram_tensor("hrowT", [F, R], fp16, kind="ExternalInput").ap()
    d_w = [nc.dram_tensor(f"w{i}", [F, F], fp16, kind="ExternalInput").ap()
           for i in (1, 2, 3)]
    d_bias = nc.dram_tensor("biasv", [1, F], f32, kind="ExternalInput").ap()
    d_attnw = nc.dram_tensor("attnw", [1, 2 * F + 2], f32, kind="ExternalInput").ap()
    d_edgew = nc.dram_tensor("edgew", [2, 2], f32, kind="ExternalInput").ap()
    d_e2nw = nc.dram_tensor("e2nw", [F, 2], f32, kind="ExternalInput").ap()
    d_idx0 = [nc.dram_tensor(f"idx0{hf}", [R, J0], i16, kind="ExternalInput").ap()
              for hf in (0, 1)]
    d_e0h = [nc.dram_tensor(f"e0h{hf}", [R, J0], fp16, kind="ExternalInput").ap()
             for hf in (0, 1)]
    d_e1h = [nc.dram_tensor(f"e1h{hf}", [R, J0], fp16, kind="ExternalInput").ap()
             for hf in (0, 1)]
    d_e0o = nc.dram_tensor("e0o", [R, J_OV], fp16, kind="ExternalInput").ap()
    d_e1o = nc.dram_tensor("e1o", [R, J_OV], fp16, kind="ExternalInput").ap()
    d_mo = nc.dram_tensor("mo", [R, J_OV], fp16, kind="ExternalInput").ap()
    d_aoff = nc.dram_tensor("aoff", [R, J_OV], i32, kind="ExternalInput").ap()
    d_zoff = nc.dram_tensor("zoff", [R, J_OV], i32, kind="ExternalInput").ap()
    d_out = nc.dram_tensor("out_rows", [R, F], f32, kind="ExternalOutput").ap()

    # internal DRAM (collective bounce buffers); one A2A carries both y's
    y12T = nc.dram_tensor("y12T", [N, R], fp16).ap()
    y12x = nc.dram_tensor("y12x", [N, R], fp16).ap()
    z_slice = nc.dram_tensor("z_slice", [R, F], fp16).ap()
    zg = nc.dram_tensor("zg", [N, F], fp16, addr_space="Shared").ap()
    ab_part = nc.dram_tensor("ab_part", [N, 1], f32).ap()
    abg = nc.dram_tensor("abg", [N, 1], f32, addr_space="Shared").ap()
    rgroups = [list(range(C))]

    with tile.TileContext(nc) as tc, tc.tile_pool(name="const", bufs=1) as cpool:
        ident = cpool.tile([P, P], fp16)
        make_identity(nc, ident[:])
        id32 = cpool.tile([P, P], f32)
        make_identity(nc, id32[:])
        ones_c16 = cpool.tile([P, 1], fp16)
        nc.vector.memset(ones_c16[:], 1.0)
        ones_r16 = cpool.tile([1, P], fp16)
        nc.vector.memset(ones_r16[:], 1.0)
        ones_r32 = cpool.tile([1, P], f32)
        nc.vector.memset(ones_r32[:], 1.0)
        ones_c32 = cpool.tile([P, 1], f32)
        nc.vector.memset(ones_c32[:], 1.0)
        ones_scat = cpool.tile([P, J0], fp16)
        nc.vector.memset(ones_scat[:], 1.0)
        bias16 = cpool.tile([1, F], fp16)
        nc.gpsimd.dma_start(out=bias16[:], in_=d_bias[:1, :])
        a1_16 = cpool.tile([1, F], fp16)
        nc.gpsimd.dma_start(out=a1_16[:], in_=d_attnw[:1, 0:F])
        a2_16 = cpool.tile([1, F], fp16)
        nc.gpsimd.dma_start(out=a2_16[:], in_=d_attnw[:1, F:2 * F])
        a1B = cpool.tile([P, F], fp16)
        a2B = cpool.tile([P, F], fp16)
        e2nT = cpool.tile([2, F], fp16)
        wa = cpool.tile([P, 4], fp16)
        biasa8 = cpool.tile([1, 2], fp16)
        edgew_sb = cpool.tile([2, 2], f32, tag="edgew")
        nc.sync.dma_start(out=edgew_sb[:2, :], in_=d_edgew[:, :])
        a3_sb = cpool.tile([2, 1], f32, tag="a3")
        nc.sync.dma_start(out=a3_sb[:2, :1], in_=d_attnw[:1, 2 * F:2 * F + 2])
        ew_row = cpool.tile([1, 4], f32, tag="ew_row")
        nc.sync.dma_start(out=ew_row[:1, :], in_=d_edgew[:, :])
        v_row = cpool.tile([1, 2], f32, tag="vrow")
        v01b = cpool.tile([P, 2], f32, tag="v01b")
        ewb = cpool.tile([P, 4], f32, tag="ewb")
        # per-core degree-derived scalars (persist across phases)
        dsum = cpool.tile([P, NT], f32)
        dinv2 = cpool.tile([P, NT], f32)
        dinv = cpool.tile([P, NT], f32)
        sqd = cpool.tile([P, NT], f32)
        dinv2b = cpool.tile([P, NT], f32)

        # ---- startup const broadcasts (PE idle here) ----
        with (
            tc.tile_pool(name="pre", bufs=1) as prep,
            tc.tile_pool(name="ps_pre", bufs=1, space="PSUM") as ps_pre,
        ):
            # v_row = a3^T @ edge_w  [1, 2]
            ps_v = ps_pre.tile([P, 2], f32, space="PSUM", tag="sm")
            nc.tensor.matmul(ps_v[:1, :2], a3_sb[:2, :1], edgew_sb[:2, :],
                             start=True, stop=True)
            nc.vector.tensor_copy(v_row[:1, :2], ps_v[:1, :2])
            ps_b1 = ps_pre.tile([P, 2], f32, space="PSUM", tag="sm")
            nc.tensor.matmul(ps_b1[:, :2], ones_r32[:1, :], v_row[:1, :2],
                             start=True, stop=True)
            nc.vector.tensor_copy(v01b[:], ps_b1[:, :2])
            ps_b2 = ps_pre.tile([P, 4], f32, space="PSUM", tag="sm")
            nc.tensor.matmul(ps_b2[:, :4], ones_r32[:1, :], ew_row[:1, :],
                             start=True, stop=True)
            nc.vector.tensor_copy(ewb[:], ps_b2[:, :4])
            # e2nT [2, F]
            for k in range(KT):
                etile = prep.tile([P, 2], fp16, tag="e2ntile")
                nc.gpsimd.dma_start(out=etile[:], in_=d_e2nw[ts(k, P), :])
                ps_t = ps_pre.tile([P, P], fp16, space="PSUM", tag="tp")
                nc.tensor.transpose(ps_t[:2, :], etile[:], ident[:])
                nc.vector.tensor_copy(e2nT[:2, ts(k, P)], ps_t[:2, :])
            # a1B/a2B broadcasts [P, F]
            for (srcv, dstv) in ((a1_16, a1B), (a2_16, a2B)):
                for chunk in range(2):
                    ps_bb = ps_pre.tile([P, 512], f32, space="PSUM", tag="bc")
                    nc.tensor.matmul(ps_bb[:], ones_r16[:1, :],
                                     srcv[:1, ts(chunk, 512)],
                                     start=True, stop=True)
                    nc.scalar.activation(dstv[:, ts(chunk, 512)],
                                         ps_bb[:], AF.Copy)

        with tc.tile_pool(name="eprep", bufs=1) as eprep:
            # edge-prep tiles that persist into the edge phase
            E0s = [eprep.tile([P, W], fp16, name=f"E0s_{b}", tag=f"E0s_{b}") for b in range(2)]
            E1s = [eprep.tile([P, W], fp16, name=f"E1s_{b}", tag=f"E1s_{b}") for b in range(2)]
            Ms = [eprep.tile([P, W], fp16, name=f"Ms_{b}", tag=f"Ms_{b}") for b in range(2)]
            aoff_t = [eprep.tile([P, J_OV], i32, name=f"aoff_{b}", tag=f"aoff_{b}") for b in range(2)]
            beta_rows = [eprep.tile([P, 1], f32, name=f"beta_{blk}",
                                    tag=f"beta_{blk}") for blk in range(2)]
            zoff_t = [eprep.tile([P, J_OV], i32, name=f"zoff_{b}", tag=f"zoff_{b}") for b in range(2)]
            for blk in range(2):
                rows_b = slice(blk * P, (blk + 1) * P)
                nc.sync.dma_start(out=aoff_t[blk][:], in_=d_aoff[rows_b, :])
                nc.sync.dma_start(out=zoff_t[blk][:], in_=d_zoff[rows_b, :])

            with tc.tile_pool(name="wts", bufs=1) as wpool:
                hcolT = wpool.tile([P, N], fp16, tag="hcolT")
                wsl = [wpool.tile([P, F], fp16, name=f"wsl_{i}", tag=f"wsl_{i}")
                       for i in range(3)]

                # =====================================================
                # Phase A: spectral part (column-sharded Chebyshev)
                # =====================================================
                with (
                    tc.tile_pool(name="adjp", bufs=1) as apool,
                    tc.tile_pool(name="awork", bufs=1) as aw,
                    tc.tile_pool(name="ps_set", bufs=1, space="PSUM") as ps_set,
                    tc.tile_pool(name="ps_a", bufs=2, space="PSUM") as ps_a,
                ):
                    _scA = nc.named_scope("phaseA"); _scA.__enter__()
                    adj_sb = [apool.tile([P, N], fp16, name=f"adj{t}", tag=f"adj{t}")
                              for t in range(NT)]
                    adj_qs = [nc.sync, nc.gpsimd, nc.scalar]
                    for t in range(NT):
                        adj_qs[t % 3].dma_start(out=adj_sb[t][:],
                                                in_=d_adj[ts(t, P), :])

                    rsc = aw.tile([P, N], fp16, tag="t_prev")  # pre-tau0 scratch
                    for t in range(NT):
                        if t % 2 == 0:
                            nc.vector.reduce_sum(dsum[:, t:t + 1], adj_sb[t][:],
                                                 axis=mybir.AxisListType.X)
                        else:
                            nc.scalar.activation(rsc[:], adj_sb[t][:],
                                                 AF.Copy,
                                                 accum_out=dsum[:, t:t + 1])
                    nc.vector.reciprocal(dinv2[:], dsum[:])
                    nc.scalar.activation(dinv[:], dinv2[:], AF.Sqrt)
                    nc.vector.tensor_tensor(out=sqd[:], in0=dsum[:], in1=dinv[:],
                                            op=ALU.mult)
                    nc.vector.tensor_scalar(out=dinv2b[:], in0=dinv2[:],
                                            scalar1=2.0 / B_CHEB, scalar2=None,
                                            op0=ALU.mult)


                    def to_row(col_t, name):
                        ps_t = ps_set.tile([NT, P], f32, space="PSUM", tag="rowt")
                        nc.tensor.transpose(ps_t[:NT, :], col_t[:, :NT], id32[:])
                        sb_t = aw.tile([NT, P], f32, tag="rowt_sb", name="rowt_sb")
                        nc.vector.tensor_copy(sb_t[:NT, :], ps_t[:NT, :])
                        row = aw.tile([1, N], fp16, tag=f"row_{name}",
                                      name=f"row_{name}")
                        nc.gpsimd.dma_start(out=row[:1, :], in_=sb_t[:NT, :])
                        return row

                    d_rowv = to_row(dsum, "d")
                    sqd_row16 = to_row(sqd, "sqd")
                    z2 = aw.tile([1, 1], f32)
                    nc.vector.reduce_sum(z2[:1, :1], d_rowv[:1, :],
                                         axis=mybir.AxisListType.X)
                    rz2 = aw.tile([1, 1], f32)
                    nc.vector.reciprocal(rz2[:1, :1], z2[:1, :1])

                    # rank-1 scalars (1/Z2) are folded into uh_row / the css
                    # copies, so the row factors need only immediate scales.
                    negd_row = aw.tile([1, N], fp16, tag="negd")
                    nc.vector.tensor_scalar(out=negd_row[:], in0=d_rowv[:],
                                            scalar1=-1.0, scalar2=None,
                                            op0=ALU.mult)
                    negd2b_row = aw.tile([1, N], fp16, tag="negd2b")
                    nc.vector.tensor_scalar(out=negd2b_row[:], in0=d_rowv[:],
                                            scalar1=-2.0 / B_CHEB, scalar2=None,
                                            op0=ALU.mult)
                    sqd_row_e4 = aw.tile([1, N], fp16, tag="sqde4")
                    nc.vector.tensor_scalar(out=sqd_row_e4[:], in0=sqd_row16[:],
                                            scalar1=float(np.exp(-4.0)),
                                            scalar2=None, op0=ALU.mult)

                    t_prev = aw.tile([P, N], fp16, tag="t_prev")
                    t_cur = aw.tile([P, N], fp16, tag="t_cur")
                    tn_tmp = aw.tile([P, N], fp16, tag="tn_tmp")
                    v_sc = aw.tile([P, N], fp16, tag="v_sc")
                    y1t = aw.tile([P, N], fp16, tag="y1t")
                    y2t = aw.tile([P, N], fp16, tag="y2t")
                    css = aw.tile([1, P], fp16, tag="css")

                    # h column slice staged through tn_tmp (reused later)
                    for t in range(NT):
                        nc.sync.dma_start(out=tn_tmp[:, ts(t, P)],
                                            in_=d_hcol[ts(t, P), :])
                    hs = aw.tile([P, N], fp16, tag="hs")
                    for t in range(NT):
                        nc.scalar.activation(hs[:, ts(t, P)], tn_tmp[:, ts(t, P)],
                                             AF.Copy, scale=sqd[:, t:t + 1])

                    ps_cs = ps_set.tile([1, P], f32, space="PSUM", tag="cs")
                    for t in range(NT):
                        nc.tensor.matmul(ps_cs[:1, :], ones_c16[:, :1],
                                         hs[:, ts(t, P)],
                                         start=(t == 0), stop=(t == NT - 1))
                    p0_row = aw.tile([1, P], f32, tag="p0")
                    nc.vector.tensor_copy(p0_row[:1, :], ps_cs[:1, :])
                    uh_row = aw.tile([1, P], fp16, tag="uh")
                    nc.vector.tensor_scalar(out=uh_row[:1, :], in0=p0_row[:1, :],
                                            scalar1=rz2[:1, :1], scalar2=None,
                                            op0=ALU.mult)
                    p0_row16 = aw.tile([1, P], fp16, tag="p016")
                    nc.vector.tensor_copy(p0_row16[:1, :], p0_row[:1, :])

                    # Software-pipelined recurrence (v/css ping-pong buffers)
                    v_nx = hs  # alias: hs is dead after tau0; reuse as 2nd v buf
                    css2 = aw.tile([1, P], fp16, tag="css2")
                    vbuf = [v_sc, v_nx]
                    csbuf = [css, css2]

                    def tail_update(dst_t, m, k):
                        """after t_{k}[m] lands: v-scale for k+1."""
                        if k == DEG:
                            return
                        nc.scalar.activation(vbuf[(k + 1) % 2][:, ts(m, P)],
                                             dst_t[:, ts(m, P)], AF.Copy,
                                             scale=dinv2b[:, m:m + 1])

                    def colsum_batch(dst_t, k):
                        """contiguous colsum group of T_k -> csbuf[(k+1)%2]."""
                        if k == DEG:
                            return
                        ps_c = ps_set.tile([1, P], f32, space="PSUM",
                                           tag="csA", name=f"ps_cs_{k}")
                        for m in range(NT):
                            nc.tensor.matmul(ps_c[:1, :], ones_c16[:, :1],
                                             dst_t[:, ts(m, P)], start=(m == 0),
                                             stop=(m == NT - 1))
                        nc.scalar.activation(csbuf[(k + 1) % 2][:1, :],
                                             ps_c[:1, :], AF.Copy,
                                             scale=rz2[:1, :1])

                    # tau0 = hs - d (1^T hs)/Z2   (k=0 stage of the pipeline)
                    for m in range(NT):
                        ps_m = ps_a.tile([P, P], f32, space="PSUM", tag="psm")
                        nc.tensor.matmul(ps_m[:], negd_row[:1, ts(m, P)],
                                         uh_row[:1, :], start=True, stop=True)
                        nc.vector.tensor_tensor(out=t_prev[:, ts(m, P)],
                                                in0=hs[:, ts(m, P)], in1=ps_m[:],
                                                op=ALU.add)
                        tail_update(t_prev, m, 0)
                    colsum_batch(t_prev, 0)
                    nc.vector.tensor_scalar(out=y1t[:], in0=t_prev[:],
                                            scalar1=float(cg[0]), scalar2=None,
                                            op0=ALU.mult)
                    nc.vector.tensor_scalar(out=y2t[:], in0=t_prev[:],
                                            scalar1=float(cf[0]), scalar2=None,
                                            op0=ALU.mult)
                    # hcolT / W-slice loads (adj DMAs have priority at start)
                    nc.gpsimd.dma_start(out=hcolT[:], in_=d_hcolT[:, :])
                    for i in range(3):
                        nc.gpsimd.dma_start(out=wsl[i][:],
                                            in_=d_wsl[ts(i, P), :])

                    # ---- edge-phase scatter prep (gpsimd idle during cheb) ----
                    for blk in range(2):
                        rows_b = slice(blk * P, (blk + 1) * P)
                        for hf in (0, 1):
                            idx_t = eprep.tile([P, J0], i16, name=f"idx_{blk}_{hf}", tag=f"idx_{blk}_{hf}")
                            nc.sync.dma_start(out=idx_t[:], in_=d_idx0[hf][rows_b, :])
                            e0_t = eprep.tile([P, J0], fp16, name=f"e0t_{blk}_{hf}", tag=f"e0t_{blk}_{hf}")
                            nc.sync.dma_start(out=e0_t[:], in_=d_e0h[hf][rows_b, :])
                            e1_t = eprep.tile([P, J0], fp16, name=f"e1t_{blk}_{hf}", tag=f"e1t_{blk}_{hf}")
                            nc.sync.dma_start(out=e1_t[:], in_=d_e1h[hf][rows_b, :])
                            nc.gpsimd.local_scatter(
                                E0s[blk][:, hf * 1024:(hf + 1) * 1024],
                                e0_t[:], idx_t[:], channels=P,
                                num_elems=1024, num_idxs=J0)
                            nc.gpsimd.local_scatter(
                                E1s[blk][:, hf * 1024:(hf + 1) * 1024],
                                e1_t[:], idx_t[:], channels=P,
                                num_elems=1024, num_idxs=J0)
                            nc.gpsimd.local_scatter(
                                Ms[blk][:, hf * 1024:(hf + 1) * 1024],
                                ones_scat[:], idx_t[:], channels=P,
                                num_elems=1024, num_idxs=J0)
                        nc.sync.dma_start(out=E0s[blk][:, N:N + J_OV],
                                          in_=d_e0o[rows_b, :])
                        nc.sync.dma_start(out=E1s[blk][:, N:N + J_OV],
                                          in_=d_e1o[rows_b, :])
                        nc.sync.dma_start(out=Ms[blk][:, N:N + J_OV],
                                          in_=d_mo[rows_b, :])
                        if W > N + J_OV:
                            nc.vector.memset(E0s[blk][:, N + J_OV:], 0.0)
                            nc.vector.memset(E1s[blk][:, N + J_OV:], 0.0)
                            nc.vector.memset(Ms[blk][:, N + J_OV:], 0.0)

                    # ---- Chebyshev recurrence ----
                    for k in range(1, DEG + 1):
                        vcur = vbuf[k % 2]
                        ccur = csbuf[k % 2]
                        dst_t = t_cur if k == 1 else t_prev
                        for m in range(NT):
                            ps_m = ps_a.tile([P, P], f32, space="PSUM", tag="psm")
                            for kk in range(NT):
                                nc.tensor.matmul(ps_m[:], adj_sb[kk][:, ts(m, P)],
                                                 vcur[:, ts(kk, P)],
                                                 start=(kk == 0), stop=False)
                            nc.tensor.matmul(ps_m[:], negd2b_row[:1, ts(m, P)],
                                             ccur[:1, :], start=False, stop=True)
                            if k == 1:
                                nc.vector.tensor_scalar(
                                    out=dst_t[:, ts(m, P)], in0=ps_m[:],
                                    scalar1=0.5, scalar2=None, op0=ALU.mult)
                            else:
                                nc.vector.scalar_tensor_tensor(
                                    out=dst_t[:, ts(m, P)], in0=ps_m[:],
                                    scalar=1.0, in1=dst_t[:, ts(m, P)],
                                    op0=ALU.mult, op1=ALU.subtract)
                            tail_update(dst_t, m, k)
                        colsum_batch(dst_t, k)
                        if k > 1:
                            t_prev, t_cur = t_cur, t_prev
                        tgt = t_cur
                        if abs(cg[k]) > 1e-7:
                            nc.vector.scalar_tensor_tensor(
                                out=y1t[:], in0=tgt[:], scalar=float(cg[k]),
                                in1=y1t[:], op0=ALU.mult, op1=ALU.add)
                        if abs(cf[k]) > 1e-7:
                            nc.vector.scalar_tensor_tensor(
                                out=y2t[:], in0=tgt[:], scalar=float(cf[k]),
                                in1=y2t[:], op0=ALU.mult, op1=ALU.add)

                    # wa[:, i] = W_i[my cols, :] @ a1 ; biasa8 = (bias @ a1)/8
                    watmp = aw.tile([P, F], fp16, tag="watmp")
                    wa32 = aw.tile([P, 4], f32, tag="wa32")
                    for i in range(3):
                        nc.vector.tensor_tensor(out=watmp[:], in0=wsl[i][:],
                                                in1=a1B[:], op=ALU.mult)
                        nc.vector.reduce_sum(wa32[:, i:i + 1], watmp[:],
                                             axis=mybir.AxisListType.X)
                    nc.vector.tensor_copy(wa[:, :3], wa32[:, :3])
                    batmp = aw.tile([1, F], fp16, tag="batmp")
                    nc.vector.tensor_tensor(out=batmp[:1, :], in0=bias16[:1, :],
                                            in1=a1_16[:1, :], op=ALU.mult)
                    bsum = aw.tile([1, 2], f32, tag="bsum")
                    nc.vector.reduce_sum(bsum[:1, 0:1], batmp[:1, :],
                                         axis=mybir.AxisListType.X)
                    nc.vector.tensor_scalar(out=biasa8[:1, 0:1],
                                            in0=bsum[:1, 0:1],
                                            scalar1=1.0 / C, scalar2=None,
                                            op0=ALU.mult)
                    # y_i = D^-1/2 y_i~ + addback*sqrt(d)(u^T h); write y^T
                    # blocks straight into the A2A layout (no phase-B
                    # transposes), and accumulate the alpha partials
                    # alpha_c = h_c@wa1 + y1_c@wa2 + y2_c@wa3 + bias.a1/8
                    y16 = v_sc
                    ytmp = tn_tmp  # tn_tmp is dead after the h staging
                    ywide = [[aw.tile([P, R], fp16, name=f"yw_{h}_{b}",
                                      tag=f"yw_{h}_{b}") for b in range(2)]
                             for h in range(2)]
                    acol = aw.tile([P, NT], f32, tag="acol")
                    for (yt, lrow, half) in ((y1t, sqd_row16, 0),
                                             (y2t, sqd_row_e4, 1)):
                        for m in range(NT):
                            ps_m = ps_a.tile([P, P], f32, space="PSUM", tag="psm")
                            nc.tensor.matmul(ps_m[:], lrow[:1, ts(m, P)],
                                             uh_row[:1, :], start=True, stop=True)
                            nc.scalar.activation(ytmp[:, ts(m, P)],
                                                 yt[:, ts(m, P)], AF.Copy,
                                                 scale=dinv[:, m:m + 1])
                            nc.vector.tensor_tensor(out=y16[:, ts(m, P)],
                                                    in0=ytmp[:, ts(m, P)],
                                                    in1=ps_m[:], op=ALU.add)
                            ps_yt = ps_set.tile([P, P], fp16, space="PSUM",
                                                tag=("ytp" if m % 2 == 0
                                                     else "ytp2"),
                                                name=f"yt_{m}_{half}")
                            nc.tensor.transpose(ps_yt[:], y16[:, ts(m, P)],
                                                ident[:])
                            yw = ywide[half][(m // 2) % 2]
                            nc.vector.tensor_copy(
                                yw[:, (m % 2) * P:(m % 2 + 1) * P], ps_yt[:])
                            if m % 2 == 1:
                                r0 = (m // 2) * R + half * P
                                nc.sync.dma_start(out=y12T[r0:r0 + P, :],
                                                  in_=yw[:])
                            ps_ab = ps_set.tile([P, 1], f32, space="PSUM",
                                                tag="pab", name=f"pab_{m}_{half}")
                            ybs = yw[:, (m % 2) * P:(m % 2 + 1) * P]
                            if half == 0:
                                nc.tensor.matmul(ps_ab[:, :1], hcolT[:, ts(m, P)],
                                                 wa[:, 0:1], start=True,
                                                 stop=False)
                                nc.tensor.matmul(ps_ab[:, :1], ybs, wa[:, 1:2],
                                                 start=False, stop=False)
                                nc.tensor.matmul(ps_ab[:, :1], ones_r16[:1, :],
                                                 biasa8[:1, 0:1], start=False,
                                                 stop=True)
                                nc.vector.tensor_copy(acol[:, m:m + 1],
                                                      ps_ab[:, :1])
                            else:
                                nc.tensor.matmul(ps_ab[:, :1], ybs, wa[:, 2:3],
                                                 start=True, stop=True)
                                nc.vector.tensor_tensor(out=acol[:, m:m + 1],
                                                        in0=acol[:, m:m + 1],
                                                        in1=ps_ab[:, :1],
                                                        op=ALU.add)
                    # acol [P, NT] -> node-flat [N, 1] via transpose + one DMA
                    ps_at = ps_set.tile([NT, P], f32, space="PSUM", tag="rowt",
                                        name="ps_at")
                    nc.tensor.transpose(ps_at[:NT, :], acol[:, :NT], id32[:])
                    acolT = aw.tile([NT, P], f32, tag="acolT")
                    nc.vector.tensor_copy(acolT[:NT, :], ps_at[:NT, :])
                    nc.sync.dma_start(out=ab_part[0:N, 0:1], in_=acolT[:NT, :])

                    _scA.__exit__(None, None, None)
                    _scC1 = nc.named_scope("a2a"); _scC1.__enter__()
                    with tc.high_priority():
                        nc.gpsimd.collective_compute(
                            "AllToAll", ALU.bypass, ins=[y12T[:]],
                            outs=[y12x[:]], replica_groups=rgroups)
                        nc.gpsimd.collective_compute(
                            "AllReduce", ALU.add, ins=[ab_part[:]],
                            outs=[abg[:]], replica_groups=rgroups)
                    _scC1.__exit__(None, None, None)

                # =====================================================
                # Phase B: z rows = h@W1 + y1@W2 + y2@W3 + bias
                # (pure matmuls; h-part runs during the A2A)
                # =====================================================
                with (
                    tc.tile_pool(name="bwork", bufs=1) as bw,
                    tc.tile_pool(name="ps_b", bufs=1, space="PSUM") as ps_b,
                ):
                    _scB = nc.named_scope("phaseB"); _scB.__enter__()
                    w_sb = [[bw.tile([P, F], fp16, name=f"w{i}_{k}",
                             tag=f"w{i}_{k}") for k in range(KT)]
                            for i in range(3)]
                    hT_sb = [bw.tile([P, R], fp16, name=f"hT_{k}",
                             tag=f"hT_{k}") for k in range(KT)]
                    for k in range(KT):
                        nc.gpsimd.dma_start(out=w_sb[0][k][:],
                                            in_=d_w[0][ts(k, P), :])
                    for k in range(KT):
                        nc.gpsimd.dma_start(out=hT_sb[k][:],
                                            in_=d_hrowT[ts(k, P), :])
                    for i in (1, 2):
                        for k in range(KT):
                            nc.gpsimd.dma_start(out=w_sb[i][k][:],
                                                in_=d_w[i][ts(k, P), :])
                    ps_h = [[ps_b.tile([P, 512], f32, space="PSUM",
                                       name=f"h{blk}{chunk}", tag=f"h{blk}{chunk}")
                              for chunk in range(2)] for blk in range(2)]
                    ps_y = [[ps_b.tile([P, 512], f32, space="PSUM",
                                       name=f"y{blk}{chunk}", tag=f"y{blk}{chunk}")
                              for chunk in range(2)] for blk in range(2)]
                    # bias + h@W1: contiguous groups, independent of the A2As
                    for blk in range(2):
                        for chunk in range(2):
                            ph = ps_h[blk][chunk]
                            nc.tensor.matmul(ph[:], ones_r16[:1, :],
                                             bias16[:1, ts(chunk, 512)],
                                             start=True, stop=False)
                            for k in range(KT):
                                nc.tensor.matmul(
                                    ph[:], hT_sb[k][:, ts(blk, P)],
                                    w_sb[0][k][:, ts(chunk, 512)],
                                    start=False, stop=(k == KT - 1))
                    # y part: per-bank contiguous groups (y1 then y2)
                    yx = [[bw.tile([P, R], fp16, name=f"yx_{yi}_{s}",
                                   tag=f"yx_{yi}_{s}") for s in range(C)]
                          for yi in range(2)]
                    for yi in range(2):
                        for s in range(C):
                            q = nc.sync if s % 2 == 0 else nc.gpsimd
                            q.dma_start(
                                out=yx[yi][s][:],
                                in_=y12x[s * R + yi * P:s * R + (yi + 1) * P, :])
                    for blk in range(2):
                        for chunk in range(2):
                            py = ps_y[blk][chunk]
                            for yi in range(2):
                                for s in range(C):
                                    nc.tensor.matmul(
                                        py[:], yx[yi][s][:, ts(blk, P)],
                                        w_sb[1 + yi][s][:, ts(chunk, 512)],
                                        start=(yi == 0 and s == 0),
                                        stop=(yi == 1 and s == C - 1))
                    for blk in range(2):
                        z16 = bw.tile([P, F], fp16, tag=f"z16_{blk}",
                                      name=f"z16_{blk}")
                        for chunk in range(2):
                            nc.scalar.activation(z16[:, ts(chunk, 512)],
                                                 ps_h[blk][chunk][:], AF.Copy)
                            nc.vector.scalar_tensor_tensor(
                                out=z16[:, ts(chunk, 512)],
                                in0=ps_y[blk][chunk][:], scalar=1.0,
                                in1=z16[:, ts(chunk, 512)],
                                op0=ALU.mult, op1=ALU.add)
                        nc.sync.dma_start(out=z_slice[ts(blk, P), :], in_=z16[:])
                        abtmp = bw.tile([P, F], fp16, tag=f"abtmp_{blk}",
                                        name=f"abtmp_{blk}")
                        nc.vector.tensor_tensor(out=abtmp[:], in0=z16[:],
                                                in1=a2B[:], op=ALU.mult)
                        nc.vector.reduce_sum(beta_rows[blk][:, 0:1], abtmp[:],
                                             axis=mybir.AxisListType.X)

                    _scB.__exit__(None, None, None)
                    _scC2 = nc.named_scope("ags"); _scC2.__enter__()
                    with tc.high_priority():
                        nc.gpsimd.collective_compute(
                            "AllGather", ALU.bypass, ins=[z_slice[:]],
                            outs=[zg[:]], replica_groups=rgroups)
                    _scC2.__exit__(None, None, None)

            # =========================================================
            # Edge phase (row-sharded dense layered softmax).
            # Softmax chain overlaps the z AllGather; z-dependent work last.
            # =========================================================
            with (
                tc.tile_pool(name="edge", bufs=1) as ep,
                tc.tile_pool(name="edge2", bufs=2) as ep2,
                tc.tile_pool(name="ps_e", bufs=2, space="PSUM") as ps_e,
            ):
                _scE = nc.named_scope("edge"); _scE.__enter__()
                # sync queue: al_row (waits ab-AG) BEFORE z_sb (waits z-AG)
                al_row = ep.tile([1, N], f32, tag="al_row")
                nc.sync.dma_start(out=al_row[:1, :], in_=abg[0:N, :1])
                # alo gathers (need only the alpha AllReduce)
                alo = [ep.tile([P, J_OV], f32, name=f"alo_{b}", tag=f"alo_{b}") for b in range(2)]
                for blk in range(2):
                    for j in range(J_OV):
                        nc.gpsimd.indirect_dma_start(
                            out=alo[blk][:, j:j + 1], out_offset=None,
                            in_=abg[:],
                            in_offset=bass.IndirectOffsetOnAxis(
                                ap=aoff_t[blk][:, j:j + 1], axis=0))
                # z tiles split across both DMA queues; zo gathers last
                z_sb = [ep.tile([P, F], fp16, name=f"z_{t}", tag=f"z_{t}")
                        for t in range(NT)]
                for t in range(NT):
                    q = nc.sync if t % 2 == 0 else nc.gpsimd
                    q.dma_start(out=z_sb[t][:], in_=zg[ts(t, P), :])
                zo_t = [[ep.tile([P, F], fp16, name=f"zo_{blk}_{j}",
                                 tag=f"zo_{blk}_{j}") for j in range(J_OV)]
                        for blk in range(2)]
                for blk in range(2):
                    for j in range(J_OV):
                        nc.gpsimd.indirect_dma_start(
                            out=zo_t[blk][j][:], out_offset=None, in_=zg[:],
                            in_offset=bass.IndirectOffsetOnAxis(
                                ap=zoff_t[blk][:, j:j + 1], axis=0))

                # vector: xp (no deps — runs during the AGs)
                xp = [ep.tile([P, W], fp16, name=f"xp_{b}", tag=f"xp_{b}") for b in range(2)]
                for blk in range(2):
                    nc.vector.tensor_scalar(out=xp[blk][:], in0=E1s[blk][:],
                                            scalar1=v01b[:, 1:2], scalar2=None,
                                            op0=ALU.mult)
                    nc.vector.scalar_tensor_tensor(out=xp[blk][:],
                                                   in0=E0s[blk][:],
                                                   scalar=v01b[:, 0:1],
                                                   in1=xp[blk][:],
                                                   op0=ALU.mult, op1=ALU.add)

                # alB broadcast (needs ab-AG)
                al_row16 = ep.tile([1, N], fp16, tag="al_row16")
                nc.vector.tensor_copy(al_row16[:1, :], al_row[:1, :])
                alB = ep.tile([P, N], fp16, tag="alB")
                for chunk in range(N // 512):
                    ps_bb = ps_e.tile([P, 512], f32, space="PSUM", tag="bc")
                    nc.tensor.matmul(ps_bb[:], ones_r16[:1, :],
                                     al_row16[:1, ts(chunk, 512)],
                                     start=True, stop=True)
                    nc.scalar.activation(alB[:, ts(chunk, 512)], ps_bb[:],
                                         AF.Copy)

                # loop 1: softmax build per blk (no z dependence)
                pmat = [ep.tile([P, W], fp16, name=f"pmat_{b}", tag=f"pmat_{b}") for b in range(2)]
                denom = [ep.tile([P, 1], f32, name=f"denom_{b}", tag=f"denom_{b}") for b in range(2)]
                qqT = [ep.tile([2, P], fp16, name=f"qqT_{b}", tag=f"qqT_{b}") for b in range(2)]
                PT = [ep.tile([P, N], fp16, name=f"PT_{b}", tag=f"PT_{b}") for b in range(2)]
                for blk in range(2):
                    beta_blk = beta_rows[blk][:, 0:1]
                    alo_b = ep2.tile([P, J_OV], f32, tag="alo_b")
                    nc.vector.tensor_scalar(out=alo_b[:], in0=alo[blk][:],
                                            scalar1=beta_blk, scalar2=None,
                                            op0=ALU.add)
                    x1 = ep2.tile([P, W], fp16, tag="x1")
                    nc.vector.scalar_tensor_tensor(out=x1[:, 0:N],
                                                   in0=xp[blk][:, 0:N],
                                                   scalar=beta_blk, in1=alB[:],
                                                   op0=ALU.add, op1=ALU.add)
                    nc.vector.tensor_copy(x1[:, N:W], xp[blk][:, N:W])
                    nc.vector.tensor_tensor(out=x1[:, N:N + J_OV],
                                            in0=xp[blk][:, N:N + J_OV],
                                            in1=alo_b[:], op=ALU.add)
                    nc.vector.scalar_tensor_tensor(out=x1[:], in0=x1[:],
                                                   scalar=0.01, in1=x1[:],
                                                   op0=ALU.mult, op1=ALU.max)
                    xm = ep2.tile([P, W], f32, tag="xm")
                    nc.vector.scalar_tensor_tensor(out=xm[:], in0=Ms[blk][:],
                                                   scalar=BIG, in1=x1[:],
                                                   op0=ALU.mult, op1=ALU.add)
                    mx = ep2.tile([P, 1], f32, tag="mx")
                    nc.vector.reduce_max(mx[:], xm[:],
                                         axis=mybir.AxisListType.X)
                    negmx = ep2.tile([P, 1], f32, tag="negmx")
                    nc.vector.tensor_scalar(out=negmx[:], in0=mx[:],
                                            scalar1=-1.0, scalar2=None,
                                            op0=ALU.mult)
                    nc.scalar.activation(pmat[blk][:], xm[:], AF.Exp,
                                         bias=negmx[:, :1],
                                         accum_out=denom[blk][:, :1])
                    s01 = ep2.tile([P, 2], f32, tag="s01")
                    x2 = ep2.tile([P, W], fp16, tag="x2")
                    for (j, Es) in ((0, E0s[blk]), (1, E1s[blk])):
                        nc.vector.scalar_tensor_tensor(
                            out=x2[:], in0=pmat[blk][:], scalar=1.0, in1=Es[:],
                            op0=ALU.mult, op1=ALU.mult,
                            accum_out=s01[:, j:j + 1])
                    q01 = ep2.tile([P, 2], fp16, tag="q01")
                    qtmp = ep2.tile([P, 1], f32, tag="qtmp")
                    for (j, ca, cb) in ((0, ewb[:, 0:1], ewb[:, 1:2]),
                                        (1, ewb[:, 2:3], ewb[:, 3:4])):
                        nc.vector.tensor_scalar(out=qtmp[:], in0=s01[:, 0:1],
                                                scalar1=ca[:, :1], scalar2=None,
                                                op0=ALU.mult)
                        nc.vector.scalar_tensor_tensor(out=q01[:, j:j + 1],
                                                       in0=s01[:, 1:2],
                                                       scalar=cb[:, :1],
                                                       in1=qtmp[:],
                                                       op0=ALU.mult, op1=ALU.add)
                    ps_q = ps_e.tile([P, P], fp16, space="PSUM", tag="tp")
                    nc.tensor.transpose(ps_q[:2, :], q01[:], ident[:])
                    nc.vector.tensor_copy(qqT[blk][:2, :], ps_q[:2, :])
                    for t in range(NT):
                        ps_t = ps_e.tile([P, P], fp16, space="PSUM", tag="tp")
                        nc.tensor.transpose(ps_t[:], pmat[blk][:, ts(t, P)],
                                            ident[:])
                        nc.vector.tensor_copy(PT[blk][:, ts(t, P)], ps_t[:])

                # loop 2: z-dependent matmuls + overflow + output
                for blk in range(2):
                    rows = slice(blk * P, (blk + 1) * P)
                    out_sb = ep2.tile([P, F], fp16, tag="out_sb")
                    for chunk in range(2):
                        ps_o = ps_e.tile([P, 512], f32, space="PSUM", tag="pso")
                        nc.tensor.matmul(ps_o[:], qqT[blk][:2, :],
                                         e2nT[:2, ts(chunk, 512)],
                                         start=True, stop=False)
                        for t in range(NT):
                            nc.tensor.matmul(ps_o[:], PT[blk][:, ts(t, P)],
                                             z_sb[t][:, ts(chunk, 512)],
                                             start=False, stop=(t == NT - 1))
                        nc.vector.tensor_copy(out_sb[:, ts(chunk, 512)],
                                              ps_o[:])

                    po32 = ep2.tile([P, J_OV], f32, tag="po32")
                    nc.vector.tensor_copy(po32[:], pmat[blk][:, N:N + J_OV])
                    for j in range(J_OV):
                        nc.vector.scalar_tensor_tensor(
                            out=out_sb[:], in0=zo_t[blk][j][:],
                            scalar=po32[:, j:j + 1], in1=out_sb[:],
                            op0=ALU.mult, op1=ALU.add)

                    recipd = ep2.tile([P, 1], f32, tag="recipd")
                    nc.vector.reciprocal(recipd[:], denom[blk][:])
                    out_f = ep2.tile([P, F], f32, tag="out_f")
                    nc.scalar.activation(out_f[:], out_sb[:], AF.Copy,
                                         scale=recipd[:, :1])
                    nc.sync.dma_start(out=d_out[rows, :], in_=out_f[:])
                _scE.__exit__(None, None, None)

    nc.compile()
    return nc


_PROGRAM_CACHE = {}


def kernel(**inputs):
    h = np.asarray(inputs["h"], np.float32)
    e = np.asarray(inputs["e"], np.float32)
    adj = np.asarray(inputs["adj"], np.float32)
    src = np.asarray(inputs["src"])
    dst = np.asarray(inputs["dst"])
    weight = np.asarray(inputs["weight"], np.float32)
    weight2 = np.asarray(inputs["weight2"], np.float32)
    weight3 = np.asarray(inputs["weight3"], np.float32)
    bias = np.asarray(inputs["bias"], np.float32)
    attn_w = np.asarray(inputs["attn_w"], np.float32)
    edge_w = np.asarray(inputs["edge_w"], np.float32)
    e2n_w = np.asarray(inputs["e2n_w"], np.float32)

    halves, J0, ov, J_OV = _host_prep(e, src, dst)
    e0o, e1o, mo, aoff, zoff = ov

    key = (J0, J_OV)
    if key not in _PROGRAM_CACHE:
        _PROGRAM_CACHE[key] = _build_program(J0, J_OV)
    nc = _PROGRAM_CACHE[key]

    adj16 = adj.astype(np.float16)
    h16 = h.astype(np.float16)
    w16 = [weight[0].astype(np.float16), weight2[0].astype(np.float16),
           weight3[0].astype(np.float16)]

    in_maps = []
    for c in range(C):
        rows = slice(c * R, (c + 1) * R)
        m = {
            "adj": adj16,
            "hcol": np.ascontiguousarray(h16[:, c * COLS:(c + 1) * COLS]),
            "hrowT": np.ascontiguousarray(h16[rows, :].T),
            "hcolT": np.ascontiguousarray(h16[:, c * COLS:(c + 1) * COLS].T),
            "wsl": np.ascontiguousarray(np.concatenate(
                [w16[i][c * COLS:(c + 1) * COLS, :] for i in range(3)])),
            "w1": w16[0], "w2": w16[1], "w3": w16[2],
            "biasv": bias.reshape(1, F),
            "attnw": attn_w.reshape(1, 2 * F + 2),
            "edgew": edge_w,
            "e2nw": e2n_w,
            "e0o": np.ascontiguousarray(e0o[rows]).astype(np.float16),
            "e1o": np.ascontiguousarray(e1o[rows]).astype(np.float16),
            "mo": np.ascontiguousarray(mo[rows]).astype(np.float16),
            "aoff": np.ascontiguousarray(aoff[rows]),
            "zoff": np.ascontiguousarray(zoff[rows]),
        }
        for hf in (0, 1):
            idx_arr, e0_arr, e1_arr = halves[hf]
            m[f"idx0{hf}"] = np.ascontiguousarray(idx_arr[rows])
            m[f"e0h{hf}"] = np.ascontiguousarray(e0_arr[rows]).astype(np.float16)
            m[f"e1h{hf}"] = np.ascontiguousarray(e1_arr[rows]).astype(np.float16)
        in_maps.append(m)

    import os
    trace = bool(os.environ.get("BASS_GNN_TRACE"))
    res = run_bass_kernel_spmd(nc, in_maps, core_ids=list(range(C)),
                               trace=trace)
    if trace:
        kernel.last_results = res
    out = np.empty((N, F), np.float32)
    for c in range(C):
        out[c * R:(c + 1) * R] = res.results[c]["out_rows"]
    return out


if __name__ == "__main__":
    D = np.load("/tmp/refdata.npz")
    inp = {k: D[k] for k in D.files if k != "expected"}
    out = kernel(**inp)
    exp = D["expected"]
    rel = np.linalg.norm(out - exp) / np.linalg.norm(exp)
    print("rel err:", rel)


# revision 44
# speedup vs baseline: 1.0084x; 1.0030x over previous
"""Trainium2 Bass kernel for nn_BlockLayer_75376676045426 (gnn_message_passing).

Math (N=2048 nodes, E=67584 edges, F=1024 features, 8 NeuronCores):
  L = I - D^-1/2 A D^-1/2,  S = D^-1/2 A D^-1/2.  The reference's
  eigh-based wavelet weights are analytic functions of S:
      w1 = exp(-2L) = g(S),   w2 = exp(-4 exp(-2L)) = f(S).
  S has the Perron pair (lambda=1, u = sqrt(d)/||sqrt(d)||) in closed form;
  after deflating it exactly, the rest of the spectrum sits inside
  [-0.4, 0.4], so w1@h, w2@h are evaluated with a single shared degree-3
  Chebyshev recurrence.
  r = h@W1 + (w1 h)@W2 + (w2 h)@W3 + bias;  then GAT-style edge softmax:
  logits_e = alpha[src] + beta[dst] + gamma_e; segment softmax over dst;
  out = P@z + rank-2 term, with the dense attention matrix P built on-chip
  via gpsimd local_scatter (multi-edge duplicates go to overflow columns).

Sharding: phase A column-parallel (adj replicated in SBUF fp16, h columns
split 8 ways); the A2A payload is written pre-transposed so phase B is
pure matmuls; phase B h@W1 part runs during the A2A; edge-phase scatter
runs during the Chebyshev recurrence; softmax overlaps the z AllGather.
"""

import sys

sys.path.insert(0, "/opt/trn_rl_repo")

import numpy as np
from numpy.polynomial import chebyshev as _cheb

import concourse.bacc as bacc
import concourse.bass as bass
import concourse.mybir as mybir
import concourse.tile as tile
from concourse.bass_utils import run_bass_kernel_spmd
from concourse.masks import make_identity

P = 128
N = 2048
F = 1024
C = 8            # cores
R = N // C       # dst rows per core (256)
NT = N // P      # 16 node tiles
KT = F // P      # 8 feature tiles
COLS = F // C    # 128 h-columns per core
B_CHEB = 0.37    # Chebyshev half-width for the bulk spectrum of S
DEG = 2
BIG = 30000.0

fp16 = mybir.dt.float16
f32 = mybir.dt.float32
i16 = mybir.dt.int16
i32 = mybir.dt.int32
AF = mybir.ActivationFunctionType
ALU = mybir.AluOpType
ts = bass.ts


def _cheb_coeffs():
    g = lambda y: np.exp(-2.0 * (1.0 - B_CHEB * y))
    f = lambda y: np.exp(-4.0 * np.exp(-2.0 * (1.0 - B_CHEB * y)))
    return (_cheb.chebinterpolate(g, DEG).astype(np.float64),
            _cheb.chebinterpolate(f, DEG).astype(np.float64))


def _host_prep(e, src, dst):
    """Index/layout-only host prep: stable sort by (dst, src), padded
    per-row scatter layouts, overflow slots for duplicate (dst, src) cells."""
    src = np.asarray(src).astype(np.int64)
    dst = np.asarray(dst).astype(np.int64)
    e = np.asarray(e)
    E = src.shape[0]
    order = np.lexsort((src, dst))
    ds, ss = dst[order], src[order]
    eo = np.ascontiguousarray(e[order])

    cell = ds * N + ss
    first = np.r_[True, cell[1:] != cell[:-1]]
    idxs = np.arange(E)
    ranks = idxs - np.maximum.accumulate(np.where(first, idxs, 0))

    l0 = ranks == 0
    J0 = 0
    for hf in (0, 1):
        sel = l0 & ((ss // 1024) == hf)
        J0 = max(J0, int(np.bincount(ds[sel], minlength=N).max()))
    J0 = (J0 + 1) // 2 * 2
    halves = []
    for hf in (0, 1):
        sel = np.where(l0 & ((ss // 1024) == hf))[0]
        idx_arr = np.full((N, J0), -1, np.int16)
        e0_arr = np.zeros((N, J0), np.float32)
        e1_arr = np.zeros((N, J0), np.float32)
        pos = np.zeros(N, np.int64)
        for k in sel:
            n = ds[k]
            j = pos[n]; pos[n] = j + 1
            idx_arr[n, j] = ss[k] - 1024 * hf
            e0_arr[n, j] = eo[k, 0]
            e1_arr[n, j] = eo[k, 1]
        halves.append((idx_arr, e0_arr, e1_arr))

    ov = np.where(ranks >= 1)[0]
    J_OV = max(2, int(np.bincount(ds[ov], minlength=N).max()) if len(ov) else 2)
    J_OV = (J_OV + 1) // 2 * 2
    e0o = np.zeros((N, J_OV), np.float32)
    e1o = np.zeros((N, J_OV), np.float32)
    mo = np.zeros((N, J_OV), np.float32)
    aoff = np.zeros((N, J_OV), np.int32)
    zoff = np.zeros((N, J_OV), np.int32)
    pos = np.zeros(N, np.int64)
    for k in ov:
        n = ds[k]
        j = pos[n]; pos[n] = j + 1
        e0o[n, j] = eo[k, 0]
        e1o[n, j] = eo[k, 1]
        mo[n, j] = 1.0
        s = int(ss[k])
        aoff[n, j] = s
        zoff[n, j] = s
    return halves, J0, (e0o, e1o, mo, aoff, zoff), J_OV


def _build_program(J0, J_OV):
    cg, cf = _cheb_coeffs()
    W = N + ((J_OV + 7) // 8) * 8
    nc = bacc.Bacc("TRN2", target_bir_lowering=False, debug=False, num_devices=C)

    # ---------------- DRAM I/O ----------------
    d_adj = nc.dram_tensor("adj", [N, N], fp16, kind="ExternalInput").ap()
    d_hcol = nc.dram_tensor("hcol", [N, COLS], fp16, kind="ExternalInput").ap()
    d_hcolT = nc.dram_tensor("hcolT", [COLS, N], fp16, kind="ExternalInput").ap()
    d_wsl = nc.dram_tensor("wsl", [3 * P, F], fp16, kind="ExternalInput").ap()
    d_hrowT = nc.dram_tensor("hrowT", [F, R], fp16, kind="ExternalInput").ap()
    d_w = [nc.dram_tensor(f"w{i}", [F, F], fp16, kind="ExternalInput").ap()
           for i in (1, 2, 3)]
    d_bias = nc.dram_tensor("biasv", [1, F], f32, kind="ExternalInput").ap()
    d_attnw = nc.dram_tensor("attnw", [1, 2 * F + 2], f32, kind="ExternalInput").ap()
    d_edgew = nc.dram_tensor("edgew", [2, 2], f32, kind="ExternalInput").ap()
    d_e2nw = nc.dram_tensor("e2nw", [F, 2], f32, kind="ExternalInput").ap()
    d_idx0 = [nc.dram_tensor(f"idx0{hf}", [R, J0], i16, kind="ExternalInput").ap()
              for hf in (0, 1)]
    d_e0h = [nc.dram_tensor(f"e0h{hf}", [R, J0], fp16, kind="ExternalInput").ap()
             for hf in (0, 1)]
    d_e1h = [nc.dram_tensor(f"e1h{hf}", [R, J0], fp16, kind="ExternalInput").ap()
             for hf in (0, 1)]
    d_e0o = nc.dram_tensor("e0o", [R, J_OV], fp16, kind="ExternalInput").ap()
    d_e1o = nc.dram_tensor("e1o", [R, J_OV], fp16, kind="ExternalInput").ap()
    d_mo = nc.dram_tensor("mo", [R, J_OV], fp16, kind="ExternalInput").ap()
    d_aoff = nc.dram_tensor("aoff", [R, J_OV], i32, kind="ExternalInput").ap()
    d_zoff = nc.dram_tensor("zoff", [R, J_OV], i32, kind="ExternalInput").ap()
    d_out = nc.dram_tensor("out_rows", [R, F], f32, kind="ExternalOutput").ap()

    # internal DRAM (collective bounce buffers); one A2A carries both y's
    y12T = nc.dram_tensor("y12T", [N, R], fp16).ap()
    y12x = nc.dram_tensor("y12x", [N, R], fp16).ap()
    z_slice = nc.dram_tensor("z_slice", [R, F], fp16).ap()
    zg = nc.dram_tensor("zg", [N, F], fp16, addr_space="Shared").ap()
    ab_part = nc.dram_tensor("ab_part", [N, 1], f32).ap()
    abg = nc.dram_tensor("abg", [N, 1], f32, addr_space="Shared").ap()
    rgroups = [list(range(C))]

    with tile.TileContext(nc) as tc, tc.tile_pool(name="const", bufs=1) as cpool:
        ident = cpool.tile([P, P], fp16)
        make_identity(nc, ident[:])
        id32 = cpool.tile([P, P], f32)
        make_identity(nc, id32[:])
        ones_c16 = cpool.tile([P, 1], fp16)
        nc.vector.memset(ones_c16[:], 1.0)
        ones_r16 = cpool.tile([1, P], fp16)
        nc.vector.memset(ones_r16[:], 1.0)
        ones_r32 = cpool.tile([1, P], f32)
        nc.vector.memset(ones_r32[:], 1.0)
        ones_c32 = cpool.tile([P, 1], f32)
        nc.vector.memset(ones_c32[:], 1.0)
        ones_scat = cpool.tile([P, J0], fp16)
        nc.vector.memset(ones_scat[:], 1.0)
        bias16 = cpool.tile([1, F], fp16)
        nc.gpsimd.dma_start(out=bias16[:], in_=d_bias[:1, :])
        a1_16 = cpool.tile([1, F], fp16)
        nc.gpsimd.dma_start(out=a1_16[:], in_=d_attnw[:1, 0:F])
        a2_16 = cpool.tile([1, F], fp16)
        nc.gpsimd.dma_start(out=a2_16[:], in_=d_attnw[:1, F:2 * F])
        a1B = cpool.tile([P, F], fp16)
        a2B = cpool.tile([P, F], fp16)
        e2nT = cpool.tile([2, F], fp16)
        wa = cpool.tile([P, 4], fp16)
        biasa8 = cpool.tile([1, 2], fp16)
        edgew_sb = cpool.tile([2, 2], f32, tag="edgew")
        nc.sync.dma_start(out=edgew_sb[:2, :], in_=d_edgew[:, :])
        a3_sb = cpool.tile([2, 1], f32, tag="a3")
        nc.sync.dma_start(out=a3_sb[:2, :1], in_=d_attnw[:1, 2 * F:2 * F + 2])
        ew_row = cpool.tile([1, 4], f32, tag="ew_row")
        nc.sync.dma_start(out=ew_row[:1, :], in_=d_edgew[:, :])
        v_row = cpool.tile([1, 2], f32, tag="vrow")
        v01b = cpool.tile([P, 2], f32, tag="v01b")
        ewb = cpool.tile([P, 4], f32, tag="ewb")
        # per-core degree-derived scalars (persist across phases)
        dsum = cpool.tile([P, NT], f32)
        dinv2 = cpool.tile([P, NT], f32)
        dinv = cpool.tile([P, NT], f32)
        sqd = cpool.tile([P, NT], f32)
        dinv2b = cpool.tile([P, NT], f32)

        # ---- startup const broadcasts (PE idle here) ----
        with (
            tc.tile_pool(name="pre", bufs=1) as prep,
            tc.tile_pool(name="ps_pre", bufs=1, space="PSUM") as ps_pre,
        ):
            # v_row = a3^T @ edge_w  [1, 2]
            ps_v = ps_pre.tile([P, 2], f32, space="PSUM", tag="sm")
            nc.tensor.matmul(ps_v[:1, :2], a3_sb[:2, :1], edgew_sb[:2, :],
                             start=True, stop=True)
            nc.vector.tensor_copy(v_row[:1, :2], ps_v[:1, :2])
            ps_b1 = ps_pre.tile([P, 2], f32, space="PSUM", tag="sm")
            nc.tensor.matmul(ps_b1[:, :2], ones_r32[:1, :], v_row[:1, :2],
                             start=True, stop=True)
            nc.vector.tensor_copy(v01b[:], ps_b1[:, :2])
            ps_b2 = ps_pre.tile([P, 4], f32, space="PSUM", tag="sm")
            nc.tensor.matmul(ps_b2[:, :4], ones_r32[:1, :], ew_row[:1, :],
                             start=True, stop=True)
            nc.vector.tensor_copy(ewb[:], ps_b2[:, :4])
            # e2nT [2, F]
            for k in range(KT):
                etile = prep.tile([P, 2], fp16, tag="e2ntile")
                nc.gpsimd.dma_start(out=etile[:], in_=d_e2nw[ts(k, P), :])
                ps_t = ps_pre.tile([P, P], fp16, space="PSUM", tag="tp")
                nc.tensor.transpose(ps_t[:2, :], etile[:], ident[:])
                nc.vector.tensor_copy(e2nT[:2, ts(k, P)], ps_t[:2, :])
            # a1B/a2B broadcasts [P, F]
            for (srcv, dstv) in ((a1_16, a1B), (a2_16, a2B)):
                for chunk in range(2):
                    ps_bb = ps_pre.tile([P, 512], f32, space="PSUM", tag="bc")
                    nc.tensor.matmul(ps_bb[:], ones_r16[:1, :],
                                     srcv[:1, ts(chunk, 512)],
                                     start=True, stop=True)
                    nc.scalar.activation(dstv[:, ts(chunk, 512)],
                                         ps_bb[:], AF.Copy)

        with tc.tile_pool(name="eprep", bufs=1) as eprep:
            # edge-prep tiles that persist into the edge phase
            E0s = [eprep.tile([P, W], fp16, name=f"E0s_{b}", tag=f"E0s_{b}") for b in range(2)]
            E1s = [eprep.tile([P, W], fp16, name=f"E1s_{b}", tag=f"E1s_{b}") for b in range(2)]
            Ms = [eprep.tile([P, W], fp16, name=f"Ms_{b}", tag=f"Ms_{b}") for b in range(2)]
            aoff_t = [eprep.tile([P, J_OV], i32, name=f"aoff_{b}", tag=f"aoff_{b}") for b in range(2)]
            beta_rows = [eprep.tile([P, 1], f32, name=f"beta_{blk}",
                                    tag=f"beta_{blk}") for blk in range(2)]
            zoff_t = [eprep.tile([P, J_OV], i32, name=f"zoff_{b}", tag=f"zoff_{b}") for b in range(2)]
            for blk in range(2):
                rows_b = slice(blk * P, (blk + 1) * P)
                nc.sync.dma_start(out=aoff_t[blk][:], in_=d_aoff[rows_b, :])
                nc.sync.dma_start(out=zoff_t[blk][:], in_=d_zoff[rows_b, :])

            with tc.tile_pool(name="wts", bufs=1) as wpool:
                hcolT = wpool.tile([P, N], fp16, tag="hcolT")
                wsl = [wpool.tile([P, F], fp16, name=f"wsl_{i}", tag=f"wsl_{i}")
                       for i in range(3)]

                # =====================================================
                # Phase A: spectral part (column-sharded Chebyshev)
                # =====================================================
                with (
                    tc.tile_pool(name="adjp", bufs=1) as apool,
                    tc.tile_pool(name="awork", bufs=1) as aw,
                    tc.tile_pool(name="ps_set", bufs=1, space="PSUM") as ps_set,
                    tc.tile_pool(name="ps_a", bufs=2, space="PSUM") as ps_a,
                ):
                    _scA = nc.named_scope("phaseA"); _scA.__enter__()
                    adj_sb = [apool.tile([P, N], fp16, name=f"adj{t}", tag=f"adj{t}")
                              for t in range(NT)]
                    adj_qs = [nc.sync, nc.gpsimd, nc.scalar]
                    for t in range(NT):
                        adj_qs[t % 3].dma_start(out=adj_sb[t][:],
                                                in_=d_adj[ts(t, P), :])

                    rsc = aw.tile([P, N], fp16, tag="t_prev")  # pre-tau0 scratch
                    for t in range(NT):
                        if t % 2 == 0:
                            nc.vector.reduce_sum(dsum[:, t:t + 1], adj_sb[t][:],
                                                 axis=mybir.AxisListType.X)
                        else:
                            nc.scalar.activation(rsc[:], adj_sb[t][:],
                                                 AF.Copy,
                                                 accum_out=dsum[:, t:t + 1])
                    nc.vector.reciprocal(dinv2[:], dsum[:])
                    nc.scalar.activation(dinv[:], dinv2[:], AF.Sqrt)
                    nc.vector.tensor_tensor(out=sqd[:], in0=dsum[:], in1=dinv[:],
                                            op=ALU.mult)
                    nc.vector.tensor_scalar(out=dinv2b[:], in0=dinv2[:],
                                            scalar1=2.0 / B_CHEB, scalar2=None,
                                            op0=ALU.mult)


                    def to_row(col_t, name):
                        ps_t = ps_set.tile([NT, P], f32, space="PSUM", tag="rowt")
                        nc.tensor.transpose(ps_t[:NT, :], col_t[:, :NT], id32[:])
                        sb_t = aw.tile([NT, P], f32, tag="rowt_sb", name="rowt_sb")
                        nc.vector.tensor_copy(sb_t[:NT, :], ps_t[:NT, :])
                        row = aw.tile([1, N], fp16, tag=f"row_{name}",
                                      name=f"row_{name}")
                        nc.gpsimd.dma_start(out=row[:1, :], in_=sb_t[:NT, :])
                        return row

                    d_rowv = to_row(dsum, "d")
                    sqd_row16 = to_row(sqd, "sqd")
                    z2 = aw.tile([1, 1], f32)
                    nc.vector.reduce_sum(z2[:1, :1], d_rowv[:1, :],
                                         axis=mybir.AxisListType.X)
                    rz2 = aw.tile([1, 1], f32)
                    nc.vector.reciprocal(rz2[:1, :1], z2[:1, :1])

                    # rank-1 scalars (1/Z2) are folded into uh_row / the css
                    # copies, so the row factors need only immediate scales.
                    negd_row = aw.tile([1, N], fp16, tag="negd")
                    nc.vector.tensor_scalar(out=negd_row[:], in0=d_rowv[:],
                                            scalar1=-1.0, scalar2=None,
                                            op0=ALU.mult)
                    negd2b_row = aw.tile([1, N], fp16, tag="negd2b")
                    nc.vector.tensor_scalar(out=negd2b_row[:], in0=d_rowv[:],
                                            scalar1=-2.0 / B_CHEB, scalar2=None,
                                            op0=ALU.mult)
                    sqd_row_e4 = aw.tile([1, N], fp16, tag="sqde4")
                    nc.vector.tensor_scalar(out=sqd_row_e4[:], in0=sqd_row16[:],
                                            scalar1=float(np.exp(-4.0)),
                                            scalar2=None, op0=ALU.mult)

                    t_prev = aw.tile([P, N], fp16, tag="t_prev")
                    t_cur = aw.tile([P, N], fp16, tag="t_cur")
                    tn_tmp = aw.tile([P, N], fp16, tag="tn_tmp")
                    v_sc = aw.tile([P, N], fp16, tag="v_sc")
                    y1t = aw.tile([P, N], fp16, tag="y1t")
                    y2t = aw.tile([P, N], fp16, tag="y2t")
                    css = aw.tile([1, P], fp16, tag="css")

                    # h column slice staged through tn_tmp (reused later)
                    for t in range(NT):
                        nc.sync.dma_start(out=tn_tmp[:, ts(t, P)],
                                            in_=d_hcol[ts(t, P), :])
                    hs = aw.tile([P, N], fp16, tag="hs")
                    for t in range(NT):
                        nc.scalar.activation(hs[:, ts(t, P)], tn_tmp[:, ts(t, P)],
                                             AF.Copy, scale=sqd[:, t:t + 1])

                    ps_cs = ps_set.tile([1, P], f32, space="PSUM", tag="cs")
                    for t in range(NT):
                        nc.tensor.matmul(ps_cs[:1, :], ones_c16[:, :1],
                                         hs[:, ts(t, P)],
                                         start=(t == 0), stop=(t == NT - 1))
                    p0_row = aw.tile([1, P], f32, tag="p0")
                    nc.vector.tensor_copy(p0_row[:1, :], ps_cs[:1, :])
                    uh_row = aw.tile([1, P], fp16, tag="uh")
                    nc.vector.tensor_scalar(out=uh_row[:1, :], in0=p0_row[:1, :],
                                            scalar1=rz2[:1, :1], scalar2=None,
                                            op0=ALU.mult)
                    p0_row16 = aw.tile([1, P], fp16, tag="p016")
                    nc.vector.tensor_copy(p0_row16[:1, :], p0_row[:1, :])

                    # Software-pipelined recurrence (v/css ping-pong buffers)
                    v_nx = hs  # alias: hs is dead after tau0; reuse as 2nd v buf
                    css2 = aw.tile([1, P], fp16, tag="css2")
                    vbuf = [v_sc, v_nx]
                    csbuf = [css, css2]

                    def tail_update(dst_t, m, k):
                        """after t_{k}[m] lands: v-scale for k+1."""
                        if k == DEG:
                            return
                        nc.scalar.activation(vbuf[(k + 1) % 2][:, ts(m, P)],
                                             dst_t[:, ts(m, P)], AF.Copy,
                                             scale=dinv2b[:, m:m + 1])

                    def colsum_batch(dst_t, k):
                        """contiguous colsum group of T_k -> csbuf[(k+1)%2]."""
                        if k == DEG:
                            return
                        ps_c = ps_set.tile([1, P], f32, space="PSUM",
                                           tag="csA", name=f"ps_cs_{k}")
                        for m in range(NT):
                            nc.tensor.matmul(ps_c[:1, :], ones_c16[:, :1],
                                             dst_t[:, ts(m, P)], start=(m == 0),
                                             stop=(m == NT - 1))
                        nc.scalar.activation(csbuf[(k + 1) % 2][:1, :],
                                             ps_c[:1, :], AF.Copy,
                                             scale=rz2[:1, :1])

                    # tau0 = hs - d (1^T hs)/Z2   (k=0 stage of the pipeline)
                    for m in range(NT):
                        ps_m = ps_a.tile([P, P], f32, space="PSUM", tag="psm")
                        nc.tensor.matmul(ps_m[:], negd_row[:1, ts(m, P)],
                                         uh_row[:1, :], start=True, stop=True)
                        nc.vector.tensor_tensor(out=t_prev[:, ts(m, P)],
                                                in0=hs[:, ts(m, P)], in1=ps_m[:],
                                                op=ALU.add)
                        tail_update(t_prev, m, 0)
                    colsum_batch(t_prev, 0)
                    nc.vector.tensor_scalar(out=y1t[:], in0=t_prev[:],
                                            scalar1=float(cg[0]), scalar2=None,
                                            op0=ALU.mult)
                    nc.vector.tensor_scalar(out=y2t[:], in0=t_prev[:],
                                            scalar1=float(cf[0]), scalar2=None,
                                            op0=ALU.mult)
                    # hcolT / W-slice loads (adj DMAs have priority at start)
                    nc.gpsimd.dma_start(out=hcolT[:], in_=d_hcolT[:, :])
                    for i in range(3):
                        nc.gpsimd.dma_start(out=wsl[i][:],
                                            in_=d_wsl[ts(i, P), :])

                    # ---- edge-phase scatter prep (gpsimd idle during cheb) ----
                    for blk in range(2):
                        rows_b = slice(blk * P, (blk + 1) * P)
                        for hf in (0, 1):
                            idx_t = eprep.tile([P, J0], i16, name=f"idx_{blk}_{hf}", tag=f"idx_{blk}_{hf}")
                            nc.sync.dma_start(out=idx_t[:], in_=d_idx0[hf][rows_b, :])
                            e0_t = eprep.tile([P, J0], fp16, name=f"e0t_{blk}_{hf}", tag=f"e0t_{blk}_{hf}")
                            nc.sync.dma_start(out=e0_t[:], in_=d_e0h[hf][rows_b, :])
                            e1_t = eprep.tile([P, J0], fp16, name=f"e1t_{blk}_{hf}", tag=f"e1t_{blk}_{hf}")
                            nc.sync.dma_start(out=e1_t[:], in_=d_e1h[hf][rows_b, :])
                            nc.gpsimd.local_scatter(
                                E0s[blk][:, hf * 1024:(hf + 1) * 1024],
                                e0_t[:], idx_t[:], channels=P,
                                num_elems=1024, num_idxs=J0)
                            nc.gpsimd.local_scatter(
                                E1s[blk][:, hf * 1024:(hf + 1) * 1024],
                                e1_t[:], idx_t[:], channels=P,
                                num_elems=1024, num_idxs=J0)
                            nc.gpsimd.local_scatter(
                                Ms[blk][:, hf * 1024:(hf + 1) * 1024],
                                ones_scat[:], idx_t[:], channels=P,
                                num_elems=1024, num_idxs=J0)
                        nc.sync.dma_start(out=E0s[blk][:, N:N + J_OV],
                                          in_=d_e0o[rows_b, :])
                        nc.sync.dma_start(out=E1s[blk][:, N:N + J_OV],
                                          in_=d_e1o[rows_b, :])
                        nc.sync.dma_start(out=Ms[blk][:, N:N + J_OV],
                                          in_=d_mo[rows_b, :])
                        if W > N + J_OV:
                            nc.vector.memset(E0s[blk][:, N + J_OV:], 0.0)
                            nc.vector.memset(E1s[blk][:, N + J_OV:], 0.0)
                            nc.vector.memset(Ms[blk][:, N + J_OV:], 0.0)

                    # ---- Chebyshev recurrence ----
                    for k in range(1, DEG + 1):
                        vcur = vbuf[k % 2]
                        ccur = csbuf[k % 2]
                        dst_t = t_cur if k == 1 else t_prev
                        for m in range(NT):
                            ps_m = ps_a.tile([P, P], f32, space="PSUM", tag="psm")
                            for kk in range(NT):
                                nc.tensor.matmul(ps_m[:], adj_sb[kk][:, ts(m, P)],
                                                 vcur[:, ts(kk, P)],
                                                 start=(kk == 0), stop=False)
                            nc.tensor.matmul(ps_m[:], negd2b_row[:1, ts(m, P)],
                                             ccur[:1, :], start=False, stop=True)
                            if k == 1:
                                nc.vector.tensor_scalar(
                                    out=dst_t[:, ts(m, P)], in0=ps_m[:],
                                    scalar1=0.5, scalar2=None, op0=ALU.mult)
                            else:
                                nc.vector.scalar_tensor_tensor(
                                    out=dst_t[:, ts(m, P)], in0=ps_m[:],
                                    scalar=1.0, in1=dst_t[:, ts(m, P)],
                                    op0=ALU.mult, op1=ALU.subtract)
                            tail_update(dst_t, m, k)
                        colsum_batch(dst_t, k)
                        if k > 1:
                            t_prev, t_cur = t_cur, t_prev
                        tgt = t_cur
                        if abs(cg[k]) > 1e-7:
                            nc.vector.scalar_tensor_tensor(
                                out=y1t[:], in0=tgt[:], scalar=float(cg[k]),
                                in1=y1t[:], op0=ALU.mult, op1=ALU.add)
                        if abs(cf[k]) > 1e-7:
                            nc.vector.scalar_tensor_tensor(
                                out=y2t[:], in0=tgt[:], scalar=float(cf[k]),
                                in1=y2t[:], op0=ALU.mult, op1=ALU.add)

                    # wa[:, i] = W_i[my cols, :] @ a1 ; biasa8 = (bias @ a1)/8
                    watmp = aw.tile([P, F], fp16, tag="watmp")
                    wa32 = aw.tile([P, 4], f32, tag="wa32")
                    for i in range(3):
                        nc.vector.tensor_tensor(out=watmp[:], in0=wsl[i][:],
                                                in1=a1B[:], op=ALU.mult)
                        nc.vector.reduce_sum(wa32[:, i:i + 1], watmp[:],
                                             axis=mybir.AxisListType.X)
                    nc.vector.tensor_copy(wa[:, :3], wa32[:, :3])
                    batmp = aw.tile([1, F], fp16, tag="batmp")
                    nc.vector.tensor_tensor(out=batmp[:1, :], in0=bias16[:1, :],
                                            in1=a1_16[:1, :], op=ALU.mult)
                    bsum = aw.tile([1, 2], f32, tag="bsum")
                    nc.vector.reduce_sum(bsum[:1, 0:1], batmp[:1, :],
                                         axis=mybir.AxisListType.X)
                    nc.vector.tensor_scalar(out=biasa8[:1, 0:1],
                                            in0=bsum[:1, 0:1],
                                            scalar1=1.0 / C, scalar2=None,
                                            op0=ALU.mult)
                    # y_i = D^-1/2 y_i~ + addback*sqrt(d)(u^T h); write y^T
                    # blocks straight into the A2A layout (no phase-B
                    # transposes), and accumulate the alpha partials
                    # alpha_c = h_c@wa1 + y1_c@wa2 + y2_c@wa3 + bias.a1/8
                    y16 = v_sc
                    ytmp = tn_tmp  # tn_tmp is dead after the h staging
                    ywide = [[aw.tile([P, R], fp16, name=f"yw_{h}_{b}",
                                      tag=f"yw_{h}_{b}") for b in range(2)]
                             for h in range(2)]
                    acol = aw.tile([P, NT], f32, tag="acol")
                    for (yt, lrow, half) in ((y1t, sqd_row16, 0),
                                             (y2t, sqd_row_e4, 1)):
                        for m in range(NT):
                            ps_m = ps_a.tile([P, P], f32, space="PSUM", tag="psm")
                            nc.tensor.matmul(ps_m[:], lrow[:1, ts(m, P)],
                                             uh_row[:1, :], start=True, stop=True)
                            nc.scalar.activation(ytmp[:, ts(m, P)],
                                                 yt[:, ts(m, P)], AF.Copy,
                                                 scale=dinv[:, m:m + 1])
                            nc.vector.tensor_tensor(out=y16[:, ts(m, P)],
                                                    in0=ytmp[:, ts(m, P)],
                                                    in1=ps_m[:], op=ALU.add)
                            ps_yt = ps_set.tile([P, P], fp16, space="PSUM",
                                                tag=("ytp" if m % 2 == 0
                                                     else "ytp2"),
                                                name=f"yt_{m}_{half}")
                            nc.tensor.transpose(ps_yt[:], y16[:, ts(m, P)],
                                                ident[:])
                            yw = ywide[half][(m // 2) % 2]
                            nc.vector.tensor_copy(
                                yw[:, (m % 2) * P:(m % 2 + 1) * P], ps_yt[:])
                            if m % 2 == 1:
                                r0 = (m // 2) * R + half * P
                                nc.sync.dma_start(out=y12T[r0:r0 + P, :],
                                                  in_=yw[:])
                            ps_ab = ps_set.tile([P, 1], f32, space="PSUM",
                                                tag="pab", name=f"pab_{m}_{half}")
                            ybs = yw[:, (m % 2) * P:(m % 2 + 1) * P]
                            if half == 0:
                                nc.tensor.matmul(ps_ab[:, :1], hcolT[:, ts(m, P)],
                                                 wa[:, 0:1], start=True,
                                                 stop=False)
                                nc.tensor.matmul(ps_ab[:, :1], ybs, wa[:, 1:2],
                                                 start=False, stop=False)
                                nc.tensor.matmul(ps_ab[:, :1], ones_r16[:1, :],
                                                 biasa8[:1, 0:1], start=False,
                                                 stop=True)
                                nc.vector.tensor_copy(acol[:, m:m + 1],
                                                      ps_ab[:, :1])
                            else:
                                nc.tensor.matmul(ps_ab[:, :1], ybs, wa[:, 2:3],
                                                 start=True, stop=True)
                                nc.vector.tensor_tensor(out=acol[:, m:m + 1],
                                                        in0=acol[:, m:m + 1],
                                                        in1=ps_ab[:, :1],
                                                        op=ALU.add)
                    # acol [P, NT] -> node-flat [N, 1] via transpose + one DMA
                    ps_at = ps_set.tile([NT, P], f32, space="PSUM", tag="rowt",
                                        name="ps_at")
                    nc.tensor.transpose(ps_at[:NT, :], acol[:, :NT], id32[:])
                    acolT = aw.tile([NT, P], f32, tag="acolT")
                    nc.vector.tensor_copy(acolT[:NT, :], ps_at[:NT, :])
                    nc.sync.dma_start(out=ab_part[0:N, 0:1], in_=acolT[:NT, :])

                    _scA.__exit__(None, None, None)
                    _scC1 = nc.named_scope("a2a"); _scC1.__enter__()
                    with tc.high_priority():
                        nc.gpsimd.collective_compute(
                            "AllToAll", ALU.bypass, ins=[y12T[:]],
                            outs=[y12x[:]], replica_groups=rgroups)
                        nc.gpsimd.collective_compute(
                            "AllReduce", ALU.add, ins=[ab_part[:]],
                            outs=[abg[:]], replica_groups=rgroups)
                    _scC1.__exit__(None, None, None)

                # =====================================================
                # Phase B: z rows = h@W1 + y1@W2 + y2@W3 + bias
                # (pure matmuls; h-part runs during the A2A)
                # =====================================================
                with (
                    tc.tile_pool(name="bwork", bufs=1) as bw,
                    tc.tile_pool(name="ps_b", bufs=1, space="PSUM") as ps_b,
                ):
                    _scB = nc.named_scope("phaseB"); _scB.__enter__()
                    w_sb = [[bw.tile([P, F], fp16, name=f"w{i}_{k}",
                             tag=f"w{i}_{k}") for k in range(KT)]
                            for i in range(3)]
                    hT_sb = [bw.tile([P, R], fp16, name=f"hT_{k}",
                             tag=f"hT_{k}") for k in range(KT)]
                    for k in range(KT):
                        nc.gpsimd.dma_start(out=w_sb[0][k][:],
                                            in_=d_w[0][ts(k, P), :])
                    for k in range(KT):
                        nc.gpsimd.dma_start(out=hT_sb[k][:],
                                            in_=d_hrowT[ts(k, P), :])
                    for i in (1, 2):
                        for k in range(KT):
                            nc.gpsimd.dma_start(out=w_sb[i][k][:],
                                                in_=d_w[i][ts(k, P), :])
                    ps_h = [[ps_b.tile([P, 512], f32, space="PSUM",
                                       name=f"h{blk}{chunk}", tag=f"h{blk}{chunk}")
                              for chunk in range(2)] for blk in range(2)]
                    ps_y = [[ps_b.tile([P, 512], f32, space="PSUM",
                                       name=f"y{blk}{chunk}", tag=f"y{blk}{chunk}")
                              for chunk in range(2)] for blk in range(2)]
                    # bias + h@W1: contiguous groups, independent of the A2As
                    for blk in range(2):
                        for chunk in range(2):
                            ph = ps_h[blk][chunk]
                            nc.tensor.matmul(ph[:], ones_r16[:1, :],
                                             bias16[:1, ts(chunk, 512)],
                                             start=True, stop=False)
                            for k in range(KT):
                                nc.tensor.matmul(
                                    ph[:], hT_sb[k][:, ts(blk, P)],
                                    w_sb[0][k][:, ts(chunk, 512)],
                                    start=False, stop=(k == KT - 1))
                    # y part: per-bank contiguous groups (y1 then y2)
                    yx = [[bw.tile([P, R], fp16, name=f"yx_{yi}_{s}",
                                   tag=f"yx_{yi}_{s}") for s in range(C)]
                          for yi in range(2)]
                    for yi in range(2):
                        for s in range(C):
                            q = nc.sync if s % 2 == 0 else nc.gpsimd
                            q.dma_start(
                                out=yx[yi][s][:],
                                in_=y12x[s * R + yi * P:s * R + (yi + 1) * P, :])
                    for blk in range(2):
                        for chunk in range(2):
                            py = ps_y[blk][chunk]
                            for yi in range(2):
                                for s in range(C):
                                    nc.tensor.matmul(
                                        py[:], yx[yi][s][:, ts(blk, P)],
                                        w_sb[1 + yi][s][:, ts(chunk, 512)],
                                        start=(yi == 0 and s == 0),
                                        stop=(yi == 1 and s == C - 1))
                    for blk in range(2):
                        z16 = bw.tile([P, F], fp16, tag=f"z16_{blk}",
                                      name=f"z16_{blk}")
                        for chunk in range(2):
                            nc.scalar.activation(z16[:, ts(chunk, 512)],
                                                 ps_h[blk][chunk][:], AF.Copy)
                            nc.vector.scalar_tensor_tensor(
                                out=z16[:, ts(chunk, 512)],
                                in0=ps_y[blk][chunk][:], scalar=1.0,
                                in1=z16[:, ts(chunk, 512)],
                                op0=ALU.mult, op1=ALU.add)
                        nc.sync.dma_start(out=z_slice[ts(blk, P), :], in_=z16[:])
                        abtmp = bw.tile([P, F], fp16, tag=f"abtmp_{blk}",
                                        name=f"abtmp_{blk}")
                        nc.vector.tensor_tensor(out=abtmp[:], in0=z16[:],
                                                in1=a2B[:], op=ALU.mult)
                        nc.vector.reduce_sum(beta_rows[blk][:, 0:1], abtmp[:],
                                             axis=mybir.AxisListType.X)

                    _scB.__exit__(None, None, None)
                    _scC2 = nc.named_scope("ags"); _scC2.__enter__()
                    with tc.high_priority():
                        nc.gpsimd.collective_compute(
                            "AllGather", ALU.bypass, ins=[z_slice[:]],
                            outs=[zg[:]], replica_groups=rgroups)
                    _scC2.__exit__(None, None, None)

            # =========================================================
            # Edge phase (row-sharded dense layered softmax).
            # Softmax chain overlaps the z AllGather; z-dependent work last.
            # =========================================================
            with (
                tc.tile_pool(name="edge", bufs=1) as ep,
                tc.tile_pool(name="edge2", bufs=2) as ep2,
                tc.tile_pool(name="ps_e", bufs=2, space="PSUM") as ps_e,
            ):
                _scE = nc.named_scope("edge"); _scE.__enter__()
                # sync queue: al_row (waits ab-AG) BEFORE z_sb (waits z-AG)
                al_row = ep.tile([1, N], f32, tag="al_row")
                nc.sync.dma_start(out=al_row[:1, :], in_=abg[0:N, :1])
                # alo gathers (need only the alpha AllReduce)
                alo = [ep.tile([P, J_OV], f32, name=f"alo_{b}", tag=f"alo_{b}") for b in range(2)]
                for blk in range(2):
                    for j in range(J_OV):
                        nc.gpsimd.indirect_dma_start(
                            out=alo[blk][:, j:j + 1], out_offset=None,
                            in_=abg[:],
                            in_offset=bass.IndirectOffsetOnAxis(
                                ap=aoff_t[blk][:, j:j + 1], axis=0))
                # z tiles split across both DMA queues; zo gathers last
                z_sb = [ep.tile([P, F], fp16, name=f"z_{t}", tag=f"z_{t}")
                        for t in range(NT)]
                for t in range(NT):
                    q = nc.sync if t % 2 == 0 else nc.gpsimd
                    q.dma_start(out=z_sb[t][:], in_=zg[ts(t, P), :])
                zo_t = [[ep.tile([P, F], fp16, name=f"zo_{blk}_{j}",
                                 tag=f"zo_{blk}_{j}") for j in range(J_OV)]
                        for blk in range(2)]
                for blk in range(2):
                    for j in range(J_OV):
                        nc.gpsimd.indirect_dma_start(
                            out=zo_t[blk][j][:], out_offset=None, in_=zg[:],
                            in_offset=bass.IndirectOffsetOnAxis(
                                ap=zoff_t[blk][:, j:j + 1], axis=0))

                # vector: xp (no deps — runs during the AGs)
                xp = [ep.tile([P, W], fp16, name=f"xp_{b}", tag=f"xp_{b}") for b in range(2)]
                for blk in range(2):
                    nc.vector.tensor_scalar(out=xp[blk][:], in0=E1s[blk][:],
                                            scalar1=v01b[:, 1:2], scalar2=None,
                                            op0=ALU.mult)
                    nc.vector.scalar_tensor_tensor(out=xp[blk][:],
                                                   in0=E0s[blk][:],
                                                   scalar=v01b[:, 0:1],
                                                   in1=xp[blk][:],
                                                   op0=ALU.mult, op1=ALU.add)

                # alB broadcast (needs ab-AG)
                al_row16 = ep.tile([1, N], fp16, tag="al_row16")
                nc.vector.tensor_copy(al_row16[:1, :], al_row[:1, :])
                alB = ep.tile([P, N], fp16, tag="alB")
                for chunk in range(N // 512):
                    ps_bb = ps_e.tile([P, 512], f32, space="PSUM", tag="bc")
                    nc.tensor.matmul(ps_bb[:], ones_r16[:1, :],
                                     al_row16[:1, ts(chunk, 512)],
                                     start=True, stop=True)
                    nc.scalar.activation(alB[:, ts(chunk, 512)], ps_bb[:],
                                         AF.Copy)

                # loop 1: softmax build per blk (no z dependence)
                pmat = [ep.tile([P, W], fp16, name=f"pmat_{b}", tag=f"pmat_{b}") for b in range(2)]
                denom = [ep.tile([P, 1], f32, name=f"denom_{b}", tag=f"denom_{b}") for b in range(2)]
                qqT = [ep.tile([2, P], fp16, name=f"qqT_{b}", tag=f"qqT_{b}") for b in range(2)]
                PT = [ep.tile([P, N], fp16, name=f"PT_{b}", tag=f"PT_{b}") for b in range(2)]
                for blk in range(2):
                    beta_blk = beta_rows[blk][:, 0:1]
                    alo_b = ep2.tile([P, J_OV], f32, tag="alo_b")
                    nc.vector.tensor_scalar(out=alo_b[:], in0=alo[blk][:],
                                            scalar1=beta_blk, scalar2=None,
                                            op0=ALU.add)
                    x1 = ep2.tile([P, W], fp16, tag="x1")
                    nc.vector.scalar_tensor_tensor(out=x1[:, 0:N],
                                                   in0=xp[blk][:, 0:N],
                                                   scalar=beta_blk, in1=alB[:],
                                                   op0=ALU.add, op1=ALU.add)
                    nc.vector.tensor_copy(x1[:, N:W], xp[blk][:, N:W])
                    nc.vector.tensor_tensor(out=x1[:, N:N + J_OV],
                                            in0=xp[blk][:, N:N + J_OV],
                                            in1=alo_b[:], op=ALU.add)
                    nc.vector.scalar_tensor_tensor(out=x1[:], in0=x1[:],
                                                   scalar=0.01, in1=x1[:],
                                                   op0=ALU.mult, op1=ALU.max)
                    xm = ep2.tile([P, W], f32, tag="xm")
                    nc.vector.scalar_tensor_tensor(out=xm[:], in0=Ms[blk][:],
                                                   scalar=BIG, in1=x1[:],
                                                   op0=ALU.mult, op1=ALU.add)
                    mx = ep2.tile([P, 1], f32, tag="mx")
                    nc.vector.reduce_max(mx[:], xm[:],
                                         axis=mybir.AxisListType.X)
                    negmx = ep2.tile([P, 1], f32, tag="negmx")
                    nc.vector.tensor_scalar(out=negmx[:], in0=mx[:],
                                            scalar1=-1.0, scalar2=None,
                                            op0=ALU.mult)
                    nc.scalar.activation(pmat[blk][:], xm[:], AF.Exp,
                                         bias=negmx[:, :1],
                                         accum_out=denom[blk][:, :1])
                    s01 = ep2.tile([P, 2], f32, tag="s01")
                    x2 = ep2.tile([P, W], fp16, tag="x2")
                    for (j, Es) in ((0, E0s[blk]), (1, E1s[blk])):
                        nc.vector.scalar_tensor_tensor(
                            out=x2[:], in0=pmat[blk][:], scalar=1.0, in1=Es[:],
                            op0=ALU.mult, op1=ALU.mult,
                            accum_out=s01[:, j:j + 1])
                    q01 = ep2.tile([P, 2], fp16, tag="q01")
                    qtmp = ep2.tile([P, 1], f32, tag="qtmp")
                    for (j, ca, cb) in ((0, ewb[:, 0:1], ewb[:, 1:2]),
                                        (1, ewb[:, 2:3], ewb[:, 3:4])):
                        nc.vector.tensor_scalar(out=qtmp[:], in0=s01[:, 0:1],
                                                scalar1=ca[:, :1], scalar2=None,
                                                op0=ALU.mult)
                        nc.vector.scalar_tensor_tensor(out=q01[:, j:j + 1],
                                                       in0=s01[:, 1:2],
                                                       scalar=cb[:, :1],
                                                       in1=qtmp[:],
                                                       op0=ALU.mult, op1=ALU.add)
                    ps_q = ps_e.tile([P, P], fp16, space="PSUM", tag="tp")
                    nc.tensor.transpose(ps_q[:2, :], q01[:], ident[:])
                    nc.vector.tensor_copy(qqT[blk][:2, :], ps_q[:2, :])
                    for t in range(NT):
                        ps_t = ps_e.tile([P, P], fp16, space="PSUM", tag="tp")
                        nc.tensor.transpose(ps_t[:], pmat[blk][:, ts(t, P)],
                                            ident[:])
                        nc.vector.tensor_copy(PT[blk][:, ts(t, P)], ps_t[:])

                # loop 2: z-dependent matmuls + overflow + output
                for blk in range(2):
                    rows = slice(blk * P, (blk + 1) * P)
                    out_sb = ep2.tile([P, F], fp16, tag="out_sb")
                    for chunk in range(2):
                        ps_o = ps_e.tile([P, 512], f32, space="PSUM", tag="pso")
                        nc.tensor.matmul(ps_o[:], qqT[blk][:2, :],
                                         e2nT[:2, ts(chunk, 512)],
                                         start=True, stop=False)
                        for t in range(NT):
                            nc.tensor.matmul(ps_o[:], PT[blk][:, ts(t, P)],
                                             z_sb[t][:, ts(chunk, 512)],
                                             start=False, stop=(t == NT - 1))
                        nc.vector.tensor_copy(out_sb[:, ts(chunk, 512)],
                                              ps_o[:])

                    po32 = ep2.tile([P, J_OV], f32, tag="po32")
                    nc.vector.tensor_copy(po32[:], pmat[blk][:, N:N + J_OV])
                    for j in range(J_OV):
                        nc.vector.scalar_tensor_tensor(
                            out=out_sb[:], in0=zo_t[blk][j][:],
                            scalar=po32[:, j:j + 1], in1=out_sb[:],
                            op0=ALU.mult, op1=ALU.add)

                    recipd = ep2.tile([P, 1], f32, tag="recipd")
                    nc.vector.reciprocal(recipd[:], denom[blk][:])
                    out_f = ep2.tile([P, F], f32, tag="out_f")
                    nc.scalar.activation(out_f[:], out_sb[:], AF.Copy,
                                         scale=recipd[:, :1])
                    nc.sync.dma_start(out=d_out[rows, :], in_=out_f[:])
                _scE.__exit__(None, None, None)

    nc.compile()
    return nc


_PROGRAM_CACHE = {}


def kernel(**inputs):
    h = np.asarray(inputs["h"], np.float32)
    e = np.asarray(inputs["e"], np.float32)
    adj = np.asarray(inputs["adj"], np.float32)
    src = np.asarray(inputs["src"])
    dst = np.asarray(inputs["dst"])
    weight = np.asarray(inputs["weight"], np.float32)
    weight2 = np.asarray(inputs["weight2"], np.float32)
    weight3 = np.asarray(inputs["weight3"], np.float32)
    bias = np.asarray(inputs["bias"], np.float32)
    attn_w = np.asarray(inputs["attn_w"], np.float32)
    edge_w = np.asarray(inputs["edge_w"], np.float32)
    e2n_w = np.asarray(inputs["e2n_w"], np.float32)

    halves, J0, ov, J_OV = _host_prep(e, src, dst)
    e0o, e1o, mo, aoff, zoff = ov

    key = (J0, J_OV)
    if key not in _PROGRAM_CACHE:
        _PROGRAM_CACHE[key] = _build_program(J0, J_OV)
    nc = _PROGRAM_CACHE[key]

    adj16 = adj.astype(np.float16)
    h16 = h.astype(np.float16)
    w16 = [weight[0].astype(np.float16), weight2[0].astype(np.float16),
           weight3[0].astype(np.float16)]

    in_maps = []
    for c in range(C):
        rows = slice(c * R, (c + 1) * R)
        m = {
            "adj": adj16,
            "hcol": np.ascontiguousarray(h16[:, c * COLS:(c + 1) * COLS]),
            "hrowT": np.ascontiguousarray(h16[rows, :].T),
            "hcolT": np.ascontiguousarray(h16[:, c * COLS:(c + 1) * COLS].T),
            "wsl": np.ascontiguousarray(np.concatenate(
                [w16[i][c * COLS:(c + 1) * COLS, :] for i in range(3)])),
            "w1": w16[0], "w2": w16[1], "w3": w16[2],
            "biasv": bias.reshape(1, F),
            "attnw": attn_w.reshape(1, 2 * F + 2),
            "edgew": edge_w,
            "e2nw": e2n_w,
            "e0o": np.ascontiguousarray(e0o[rows]).astype(np.float16),
            "e1o": np.ascontiguousarray(e1o[rows]).astype(np.float16),
            "mo": np.ascontiguousarray(mo[rows]).astype(np.float16),
            "aoff": np.ascontiguousarray(aoff[rows]),
            "zoff": np.ascontiguousarray(zoff[rows]),
        }
        for hf in (0, 1):
            idx_arr, e0_arr, e1_arr = halves[hf]
            m[f"idx0{hf}"] = np.ascontiguousarray(idx_arr[rows])
            m[f"e0h{hf}"] = np.ascontiguousarray(e0_arr[rows]).astype(np.float16)
            m[f"e1h{hf}"] = np.ascontiguousarray(e1_arr[rows]).astype(np.float16)
        in_maps.append(m)

    import os
    trace = bool(os.environ.get("BASS_GNN_TRACE"))
    res = run_bass_kernel_spmd(nc, in_maps, core_ids=list(range(C)),
                               trace=trace)
    if trace:
        kernel.last_results = res
    out = np.empty((N, F), np.float32)
    for c in range(C):
        out[c * R:(c + 1) * R] = res.results[c]["out_rows"]
    return out


if __name__ == "__main__":
    D = np.load("/tmp/refdata.npz")
    inp = {k: D[k] for k in D.files if k != "expected"}
    out = kernel(**inp)
    exp = D["expected"]
    rel = np.linalg.norm(out - exp) / np.linalg.norm(exp)
    print("rel err:", rel)


# revision 45
# speedup vs baseline: 1.0672x; 1.0584x over previous
"""Trainium2 Bass kernel for nn_BlockLayer_75376676045426 (gnn_message_passing).

Math (N=2048 nodes, E=67584 edges, F=1024 features, 8 NeuronCores):
  L = I - D^-1/2 A D^-1/2,  S = D^-1/2 A D^-1/2.  The reference's
  eigh-based wavelet weights are analytic functions of S:
      w1 = exp(-2L) = g(S),   w2 = exp(-4 exp(-2L)) = f(S).
  S has the Perron pair (lambda=1, u = sqrt(d)/||sqrt(d)||) in closed form;
  after deflating it exactly, the rest of the spectrum sits inside
  [-0.4, 0.4], so w1@h, w2@h are evaluated with a single shared degree-3
  Chebyshev recurrence.
  r = h@W1 + (w1 h)@W2 + (w2 h)@W3 + bias;  then GAT-style edge softmax:
  logits_e = alpha[src] + beta[dst] + gamma_e; segment softmax over dst;
  out = P@z + rank-2 term, with the dense attention matrix P built on-chip
  via gpsimd local_scatter (multi-edge duplicates go to overflow columns).

Sharding: phase A column-parallel (adj replicated in SBUF fp16, h columns
split 8 ways); the A2A payload is written pre-transposed so phase B is
pure matmuls; phase B h@W1 part runs during the A2A; edge-phase scatter
runs during the Chebyshev recurrence; softmax overlaps the z AllGather.
"""

import sys

sys.path.insert(0, "/opt/trn_rl_repo")

import numpy as np
from numpy.polynomial import chebyshev as _cheb

import concourse.bacc as bacc
import concourse.bass as bass
import concourse.mybir as mybir
import concourse.tile as tile
from concourse.bass_utils import run_bass_kernel_spmd
from concourse.masks import make_identity

P = 128
N = 2048
F = 1024
C = 8            # cores
R = N // C       # dst rows per core (256)
NT = N // P      # 16 node tiles
KT = F // P      # 8 feature tiles
COLS = F // C    # 128 h-columns per core
B_CHEB = 0.37    # Chebyshev half-width for the bulk spectrum of S
DEG = 2
BIG = 30000.0

fp16 = mybir.dt.float16
f32 = mybir.dt.float32
i16 = mybir.dt.int16
i32 = mybir.dt.int32
AF = mybir.ActivationFunctionType
ALU = mybir.AluOpType
ts = bass.ts


def _cheb_coeffs():
    g = lambda y: np.exp(-2.0 * (1.0 - B_CHEB * y))
    f = lambda y: np.exp(-4.0 * np.exp(-2.0 * (1.0 - B_CHEB * y)))
    return (_cheb.chebinterpolate(g, DEG).astype(np.float64),
            _cheb.chebinterpolate(f, DEG).astype(np.float64))


def _host_prep(e, src, dst):
    """Index/layout-only host prep: stable sort by (dst, src), padded
    per-row scatter layouts, overflow slots for duplicate (dst, src) cells."""
    src = np.asarray(src).astype(np.int64)
    dst = np.asarray(dst).astype(np.int64)
    e = np.asarray(e)
    E = src.shape[0]
    order = np.lexsort((src, dst))
    ds, ss = dst[order], src[order]
    eo = np.ascontiguousarray(e[order])

    cell = ds * N + ss
    first = np.r_[True, cell[1:] != cell[:-1]]
    idxs = np.arange(E)
    ranks = idxs - np.maximum.accumulate(np.where(first, idxs, 0))

    l0 = ranks == 0
    J0 = 0
    for hf in (0, 1):
        sel = l0 & ((ss // 1024) == hf)
        J0 = max(J0, int(np.bincount(ds[sel], minlength=N).max()))
    J0 = (J0 + 1) // 2 * 2
    halves = []
    for hf in (0, 1):
        sel = np.where(l0 & ((ss // 1024) == hf))[0]
        idx_arr = np.full((N, J0), -1, np.int16)
        e0_arr = np.zeros((N, J0), np.float32)
        e1_arr = np.zeros((N, J0), np.float32)
        pos = np.zeros(N, np.int64)
        for k in sel:
            n = ds[k]
            j = pos[n]; pos[n] = j + 1
            idx_arr[n, j] = ss[k] - 1024 * hf
            e0_arr[n, j] = eo[k, 0]
            e1_arr[n, j] = eo[k, 1]
        halves.append((idx_arr, e0_arr, e1_arr))

    ov = np.where(ranks >= 1)[0]
    J_OV = max(2, int(np.bincount(ds[ov], minlength=N).max()) if len(ov) else 2)
    J_OV = (J_OV + 1) // 2 * 2
    e0o = np.zeros((N, J_OV), np.float32)
    e1o = np.zeros((N, J_OV), np.float32)
    mo = np.zeros((N, J_OV), np.float32)
    aoff = np.zeros((N, J_OV), np.int32)
    zoff = np.zeros((N, J_OV), np.int32)
    pos = np.zeros(N, np.int64)
    for k in ov:
        n = ds[k]
        j = pos[n]; pos[n] = j + 1
        e0o[n, j] = eo[k, 0]
        e1o[n, j] = eo[k, 1]
        mo[n, j] = 1.0
        s = int(ss[k])
        aoff[n, j] = s
        zoff[n, j] = s
    return halves, J0, (e0o, e1o, mo, aoff, zoff), J_OV


def _build_program(J0, J_OV):
    cg, cf = _cheb_coeffs()
    W = N + ((J_OV + 7) // 8) * 8
    nc = bacc.Bacc("TRN2", target_bir_lowering=False, debug=False, num_devices=C)

    # ---------------- DRAM I/O ----------------
    d_adj = nc.dram_tensor("adj", [N, N], fp16, kind="ExternalInput").ap()
    d_hcol = nc.dram_tensor("hcol", [N, COLS], fp16, kind="ExternalInput").ap()
    d_hcolT = nc.dram_tensor("hcolT", [COLS, N], fp16, kind="ExternalInput").ap()
    d_wsl = nc.dram_tensor("wsl", [3 * P, F], fp16, kind="ExternalInput").ap()
    d_hrowT = nc.dram_tensor("hrowT", [F, R], fp16, kind="ExternalInput").ap()
    d_w = [nc.dram_tensor(f"w{i}", [F, F], fp16, kind="ExternalInput").ap()
           for i in (1, 2, 3)]
    d_bias = nc.dram_tensor("biasv", [1, F], f32, kind="ExternalInput").ap()
    d_attnw = nc.dram_tensor("attnw", [1, 2 * F + 2], f32, kind="ExternalInput").ap()
    d_edgew = nc.dram_tensor("edgew", [2, 2], f32, kind="ExternalInput").ap()
    d_e2nw = nc.dram_tensor("e2nw", [F, 2], f32, kind="ExternalInput").ap()
    d_idx0 = [nc.dram_tensor(f"idx0{hf}", [R, J0], i16, kind="ExternalInput").ap()
              for hf in (0, 1)]
    d_e0h = [nc.dram_tensor(f"e0h{hf}", [R, J0], fp16, kind="ExternalInput").ap()
             for hf in (0, 1)]
    d_e1h = [nc.dram_tensor(f"e1h{hf}", [R, J0], fp16, kind="ExternalInput").ap()
             for hf in (0, 1)]
    d_e0o = nc.dram_tensor("e0o", [R, J_OV], fp16, kind="ExternalInput").ap()
    d_e1o = nc.dram_tensor("e1o", [R, J_OV], fp16, kind="ExternalInput").ap()
    d_mo = nc.dram_tensor("mo", [R, J_OV], fp16, kind="ExternalInput").ap()
    d_aoff = nc.dram_tensor("aoff", [R, J_OV], i32, kind="ExternalInput").ap()
    d_zoff = nc.dram_tensor("zoff", [R, J_OV], i32, kind="ExternalInput").ap()
    d_out = nc.dram_tensor("out_rows", [R, F], f32, kind="ExternalOutput").ap()

    # internal DRAM (collective bounce buffers); one A2A carries both y's
    y12T = nc.dram_tensor("y12T", [N, R], fp16).ap()
    y12x = nc.dram_tensor("y12x", [N, R], fp16).ap()
    z_slice = nc.dram_tensor("z_slice", [R, F], fp16).ap()
    zg = nc.dram_tensor("zg", [N, F], fp16, addr_space="Shared").ap()
    ab_part = nc.dram_tensor("ab_part", [N, 1], f32).ap()
    abg = nc.dram_tensor("abg", [N, 1], f32, addr_space="Shared").ap()
    rgroups = [list(range(C))]

    with tile.TileContext(nc) as tc, tc.tile_pool(name="const", bufs=1) as cpool:
        ident = cpool.tile([P, P], fp16)
        make_identity(nc, ident[:])
        id32 = cpool.tile([P, P], f32)
        make_identity(nc, id32[:])
        ones_c16 = cpool.tile([P, 1], fp16)
        nc.vector.memset(ones_c16[:], 1.0)
        ones_r16 = cpool.tile([1, P], fp16)
        nc.vector.memset(ones_r16[:], 1.0)
        ones_r32 = cpool.tile([1, P], f32)
        nc.vector.memset(ones_r32[:], 1.0)
        ones_c32 = cpool.tile([P, 1], f32)
        nc.vector.memset(ones_c32[:], 1.0)
        ones_scat = cpool.tile([P, J0], fp16)
        nc.vector.memset(ones_scat[:], 1.0)
        bias16 = cpool.tile([1, F], fp16)
        nc.gpsimd.dma_start(out=bias16[:], in_=d_bias[:1, :])
        a1_16 = cpool.tile([1, F], fp16)
        nc.gpsimd.dma_start(out=a1_16[:], in_=d_attnw[:1, 0:F])
        a2_16 = cpool.tile([1, F], fp16)
        nc.gpsimd.dma_start(out=a2_16[:], in_=d_attnw[:1, F:2 * F])
        a1B = cpool.tile([P, F], fp16)
        a2B = cpool.tile([P, F], fp16)
        e2nT = cpool.tile([2, F], fp16)
        wa = cpool.tile([P, 4], fp16)
        biasa8 = cpool.tile([1, 2], fp16)
        edgew_sb = cpool.tile([2, 2], f32, tag="edgew")
        nc.sync.dma_start(out=edgew_sb[:2, :], in_=d_edgew[:, :])
        a3_sb = cpool.tile([2, 1], f32, tag="a3")
        nc.sync.dma_start(out=a3_sb[:2, :1], in_=d_attnw[:1, 2 * F:2 * F + 2])
        ew_row = cpool.tile([1, 4], f32, tag="ew_row")
        nc.sync.dma_start(out=ew_row[:1, :], in_=d_edgew[:, :])
        v_row = cpool.tile([1, 2], f32, tag="vrow")
        v01b = cpool.tile([P, 2], f32, tag="v01b")
        ewb = cpool.tile([P, 4], f32, tag="ewb")
        # per-core degree-derived scalars (persist across phases)
        dsum = cpool.tile([P, NT], f32)
        dinv2 = cpool.tile([P, NT], f32)
        dinv = cpool.tile([P, NT], f32)
        sqd = cpool.tile([P, NT], f32)
        dinv2b = cpool.tile([P, NT], f32)

        # ---- startup const broadcasts (PE idle here) ----
        with (
            tc.tile_pool(name="pre", bufs=1) as prep,
            tc.tile_pool(name="ps_pre", bufs=1, space="PSUM") as ps_pre,
        ):
            # v_row = a3^T @ edge_w  [1, 2]
            ps_v = ps_pre.tile([P, 2], f32, space="PSUM", tag="sm")
            nc.tensor.matmul(ps_v[:1, :2], a3_sb[:2, :1], edgew_sb[:2, :],
                             start=True, stop=True)
            nc.vector.tensor_copy(v_row[:1, :2], ps_v[:1, :2])
            ps_b1 = ps_pre.tile([P, 2], f32, space="PSUM", tag="sm")
            nc.tensor.matmul(ps_b1[:, :2], ones_r32[:1, :], v_row[:1, :2],
                             start=True, stop=True)
            nc.vector.tensor_copy(v01b[:], ps_b1[:, :2])
            ps_b2 = ps_pre.tile([P, 4], f32, space="PSUM", tag="sm")
            nc.tensor.matmul(ps_b2[:, :4], ones_r32[:1, :], ew_row[:1, :],
                             start=True, stop=True)
            nc.vector.tensor_copy(ewb[:], ps_b2[:, :4])
            # e2nT [2, F]
            for k in range(KT):
                etile = prep.tile([P, 2], fp16, tag="e2ntile")
                nc.gpsimd.dma_start(out=etile[:], in_=d_e2nw[ts(k, P), :])
                ps_t = ps_pre.tile([P, P], fp16, space="PSUM", tag="tp")
                nc.tensor.transpose(ps_t[:2, :], etile[:], ident[:])
                nc.vector.tensor_copy(e2nT[:2, ts(k, P)], ps_t[:2, :])
            # a1B/a2B broadcasts [P, F]
            for (srcv, dstv) in ((a1_16, a1B), (a2_16, a2B)):
                for chunk in range(2):
                    ps_bb = ps_pre.tile([P, 512], f32, space="PSUM", tag="bc")
                    nc.tensor.matmul(ps_bb[:], ones_r16[:1, :],
                                     srcv[:1, ts(chunk, 512)],
                                     start=True, stop=True)
                    nc.scalar.activation(dstv[:, ts(chunk, 512)],
                                         ps_bb[:], AF.Copy)

        with tc.tile_pool(name="eprep", bufs=1) as eprep:
            # edge-prep tiles that persist into the edge phase
            E0s = [eprep.tile([P, W], fp16, name=f"E0s_{b}", tag=f"E0s_{b}") for b in range(2)]
            E1s = [eprep.tile([P, W], fp16, name=f"E1s_{b}", tag=f"E1s_{b}") for b in range(2)]
            Ms = [eprep.tile([P, W], fp16, name=f"Ms_{b}", tag=f"Ms_{b}") for b in range(2)]
            aoff_t = [eprep.tile([P, J_OV], i32, name=f"aoff_{b}", tag=f"aoff_{b}") for b in range(2)]
            beta_rows = [eprep.tile([P, 1], f32, name=f"beta_{blk}",
                                    tag=f"beta_{blk}") for blk in range(2)]
            zoff_t = [eprep.tile([P, J_OV], i32, name=f"zoff_{b}", tag=f"zoff_{b}") for b in range(2)]
            for blk in range(2):
                rows_b = slice(blk * P, (blk + 1) * P)
                nc.sync.dma_start(out=aoff_t[blk][:], in_=d_aoff[rows_b, :])
                nc.sync.dma_start(out=zoff_t[blk][:], in_=d_zoff[rows_b, :])

            with tc.tile_pool(name="wts", bufs=1) as wpool:
                hcolT = wpool.tile([P, N], fp16, tag="hcolT")
                wsl = [wpool.tile([P, F], fp16, name=f"wsl_{i}", tag=f"wsl_{i}")
                       for i in range(3)]

                # =====================================================
                # Phase A: spectral part (column-sharded Chebyshev)
                # =====================================================
                with (
                    tc.tile_pool(name="adjp", bufs=1) as apool,
                    tc.tile_pool(name="awork", bufs=1) as aw,
                    tc.tile_pool(name="ps_set", bufs=1, space="PSUM") as ps_set,
                    tc.tile_pool(name="ps_a", bufs=2, space="PSUM") as ps_a,
                ):
                    _scA = nc.named_scope("phaseA"); _scA.__enter__()
                    adj_sb = [apool.tile([P, N], fp16, name=f"adj{t}", tag=f"adj{t}")
                              for t in range(NT)]
                    adj_qs = [nc.sync, nc.gpsimd, nc.scalar]
                    for t in range(NT):
                        adj_qs[t % 3].dma_start(out=adj_sb[t][:],
                                                in_=d_adj[ts(t, P), :])

                    rsc = aw.tile([P, N], fp16, tag="t_prev")  # pre-tau0 scratch
                    for t in range(NT):
                        if t % 2 == 0:
                            nc.vector.reduce_sum(dsum[:, t:t + 1], adj_sb[t][:],
                                                 axis=mybir.AxisListType.X)
                        else:
                            nc.scalar.activation(rsc[:], adj_sb[t][:],
                                                 AF.Copy,
                                                 accum_out=dsum[:, t:t + 1])
                    nc.vector.reciprocal(dinv2[:], dsum[:])
                    nc.scalar.activation(dinv[:], dinv2[:], AF.Sqrt)
                    nc.vector.tensor_tensor(out=sqd[:], in0=dsum[:], in1=dinv[:],
                                            op=ALU.mult)
                    nc.vector.tensor_scalar(out=dinv2b[:], in0=dinv2[:],
                                            scalar1=2.0 / B_CHEB, scalar2=None,
                                            op0=ALU.mult)


                    def to_row(col_t, name):
                        ps_t = ps_set.tile([NT, P], f32, space="PSUM", tag="rowt")
                        nc.tensor.transpose(ps_t[:NT, :], col_t[:, :NT], id32[:])
                        sb_t = aw.tile([NT, P], f32, tag="rowt_sb", name="rowt_sb")
                        nc.vector.tensor_copy(sb_t[:NT, :], ps_t[:NT, :])
                        row = aw.tile([1, N], fp16, tag=f"row_{name}",
                                      name=f"row_{name}")
                        nc.gpsimd.dma_start(out=row[:1, :], in_=sb_t[:NT, :])
                        return row

                    d_rowv = to_row(dsum, "d")
                    sqd_row16 = to_row(sqd, "sqd")
                    z2 = aw.tile([1, 1], f32)
                    nc.vector.reduce_sum(z2[:1, :1], d_rowv[:1, :],
                                         axis=mybir.AxisListType.X)
                    rz2 = aw.tile([1, 1], f32)
                    nc.vector.reciprocal(rz2[:1, :1], z2[:1, :1])

                    # rank-1 scalars (1/Z2) are folded into uh_row / the css
                    # copies, so the row factors need only immediate scales.
                    negd_row = aw.tile([1, N], fp16, tag="negd")
                    nc.vector.tensor_scalar(out=negd_row[:], in0=d_rowv[:],
                                            scalar1=-1.0, scalar2=None,
                                            op0=ALU.mult)
                    negd2b_row = aw.tile([1, N], fp16, tag="negd2b")
                    nc.vector.tensor_scalar(out=negd2b_row[:], in0=d_rowv[:],
                                            scalar1=-2.0 / B_CHEB, scalar2=None,
                                            op0=ALU.mult)
                    sqd_row_e4 = aw.tile([1, N], fp16, tag="sqde4")
                    nc.vector.tensor_scalar(out=sqd_row_e4[:], in0=sqd_row16[:],
                                            scalar1=float(np.exp(-4.0)),
                                            scalar2=None, op0=ALU.mult)

                    t_prev = aw.tile([P, N], fp16, tag="t_prev")
                    t_cur = aw.tile([P, N], fp16, tag="t_cur")
                    tn_tmp = aw.tile([P, N], fp16, tag="tn_tmp")
                    v_sc = aw.tile([P, N], fp16, tag="v_sc")
                    y1t = aw.tile([P, N], fp16, tag="y1t")
                    y2t = aw.tile([P, N], fp16, tag="y2t")
                    css = aw.tile([1, P], fp16, tag="css")

                    # h column slice staged through tn_tmp (reused later)
                    for t in range(NT):
                        nc.sync.dma_start(out=tn_tmp[:, ts(t, P)],
                                            in_=d_hcol[ts(t, P), :])
                    hs = aw.tile([P, N], fp16, tag="hs")
                    for t in range(NT):
                        nc.scalar.activation(hs[:, ts(t, P)], tn_tmp[:, ts(t, P)],
                                             AF.Copy, scale=sqd[:, t:t + 1])

                    ps_cs = ps_set.tile([1, P], f32, space="PSUM", tag="cs")
                    for t in range(NT):
                        nc.tensor.matmul(ps_cs[:1, :], ones_c16[:, :1],
                                         hs[:, ts(t, P)],
                                         start=(t == 0), stop=(t == NT - 1))
                    p0_row = aw.tile([1, P], f32, tag="p0")
                    nc.vector.tensor_copy(p0_row[:1, :], ps_cs[:1, :])
                    uh_row = aw.tile([1, P], fp16, tag="uh")
                    nc.vector.tensor_scalar(out=uh_row[:1, :], in0=p0_row[:1, :],
                                            scalar1=rz2[:1, :1], scalar2=None,
                                            op0=ALU.mult)
                    p0_row16 = aw.tile([1, P], fp16, tag="p016")
                    nc.vector.tensor_copy(p0_row16[:1, :], p0_row[:1, :])

                    # Software-pipelined recurrence (v/css ping-pong buffers)
                    v_nx = hs  # alias: hs is dead after tau0; reuse as 2nd v buf
                    css2 = aw.tile([1, P], fp16, tag="css2")
                    vbuf = [v_sc, v_nx]
                    csbuf = [css, css2]

                    def tail_update(dst_t, m, k):
                        """after t_{k}[m] lands: v-scale for k+1."""
                        if k == DEG:
                            return
                        nc.scalar.activation(vbuf[(k + 1) % 2][:, ts(m, P)],
                                             dst_t[:, ts(m, P)], AF.Copy,
                                             scale=dinv2b[:, m:m + 1])

                    def colsum_batch(dst_t, k):
                        """contiguous colsum group of T_k -> csbuf[(k+1)%2]."""
                        if k == DEG:
                            return
                        ps_c = ps_set.tile([1, P], f32, space="PSUM",
                                           tag="csA", name=f"ps_cs_{k}")
                        for m in range(NT):
                            nc.tensor.matmul(ps_c[:1, :], ones_c16[:, :1],
                                             dst_t[:, ts(m, P)], start=(m == 0),
                                             stop=(m == NT - 1))
                        nc.scalar.activation(csbuf[(k + 1) % 2][:1, :],
                                             ps_c[:1, :], AF.Copy,
                                             scale=rz2[:1, :1])

                    # tau0 = hs - d (1^T hs)/Z2   (k=0 stage of the pipeline)
                    for m in range(NT):
                        ps_m = ps_a.tile([P, P], f32, space="PSUM", tag="psm")
                        nc.tensor.matmul(ps_m[:], negd_row[:1, ts(m, P)],
                                         uh_row[:1, :], start=True, stop=True)
                        nc.vector.tensor_tensor(out=t_prev[:, ts(m, P)],
                                                in0=hs[:, ts(m, P)], in1=ps_m[:],
                                                op=ALU.add)
                        tail_update(t_prev, m, 0)
                    colsum_batch(t_prev, 0)
                    nc.vector.tensor_scalar(out=y1t[:], in0=t_prev[:],
                                            scalar1=float(cg[0]), scalar2=None,
                                            op0=ALU.mult)
                    nc.vector.tensor_scalar(out=y2t[:], in0=t_prev[:],
                                            scalar1=float(cf[0]), scalar2=None,
                                            op0=ALU.mult)
                    # hcolT / W-slice loads (adj DMAs have priority at start)
                    nc.gpsimd.dma_start(out=hcolT[:], in_=d_hcolT[:, :])
                    for i in range(3):
                        nc.gpsimd.dma_start(out=wsl[i][:],
                                            in_=d_wsl[ts(i, P), :])

                    # ---- edge-phase scatter prep (gpsimd idle during cheb) ----
                    for blk in range(2):
                        rows_b = slice(blk * P, (blk + 1) * P)
                        for hf in (0, 1):
                            idx_t = eprep.tile([P, J0], i16, name=f"idx_{blk}_{hf}", tag=f"idx_{blk}_{hf}")
                            nc.sync.dma_start(out=idx_t[:], in_=d_idx0[hf][rows_b, :])
                            e0_t = eprep.tile([P, J0], fp16, name=f"e0t_{blk}_{hf}", tag=f"e0t_{blk}_{hf}")
                            nc.sync.dma_start(out=e0_t[:], in_=d_e0h[hf][rows_b, :])
                            e1_t = eprep.tile([P, J0], fp16, name=f"e1t_{blk}_{hf}", tag=f"e1t_{blk}_{hf}")
                            nc.sync.dma_start(out=e1_t[:], in_=d_e1h[hf][rows_b, :])
                            nc.gpsimd.local_scatter(
                                E0s[blk][:, hf * 1024:(hf + 1) * 1024],
                                e0_t[:], idx_t[:], channels=P,
                                num_elems=1024, num_idxs=J0)
                            nc.gpsimd.local_scatter(
                                E1s[blk][:, hf * 1024:(hf + 1) * 1024],
                                e1_t[:], idx_t[:], channels=P,
                                num_elems=1024, num_idxs=J0)
                            nc.gpsimd.local_scatter(
                                Ms[blk][:, hf * 1024:(hf + 1) * 1024],
                                ones_scat[:], idx_t[:], channels=P,
                                num_elems=1024, num_idxs=J0)
                        nc.sync.dma_start(out=E0s[blk][:, N:N + J_OV],
                                          in_=d_e0o[rows_b, :])
                        nc.sync.dma_start(out=E1s[blk][:, N:N + J_OV],
                                          in_=d_e1o[rows_b, :])
                        nc.sync.dma_start(out=Ms[blk][:, N:N + J_OV],
                                          in_=d_mo[rows_b, :])
                        if W > N + J_OV:
                            nc.vector.memset(E0s[blk][:, N + J_OV:], 0.0)
                            nc.vector.memset(E1s[blk][:, N + J_OV:], 0.0)
                            nc.vector.memset(Ms[blk][:, N + J_OV:], 0.0)

                    # ---- Chebyshev recurrence ----
                    for k in range(1, DEG + 1):
                        vcur = vbuf[k % 2]
                        ccur = csbuf[k % 2]
                        dst_t = t_cur if k == 1 else t_prev
                        for m in range(NT):
                            ps_m = ps_a.tile([P, P], f32, space="PSUM", tag="psm")
                            for kk in range(NT):
                                nc.tensor.matmul(ps_m[:], adj_sb[kk][:, ts(m, P)],
                                                 vcur[:, ts(kk, P)],
                                                 start=(kk == 0), stop=False)
                            nc.tensor.matmul(ps_m[:], negd2b_row[:1, ts(m, P)],
                                             ccur[:1, :], start=False, stop=True)
                            if k == 1:
                                nc.vector.tensor_scalar(
                                    out=dst_t[:, ts(m, P)], in0=ps_m[:],
                                    scalar1=0.5, scalar2=None, op0=ALU.mult)
                            else:
                                nc.vector.scalar_tensor_tensor(
                                    out=dst_t[:, ts(m, P)], in0=ps_m[:],
                                    scalar=1.0, in1=dst_t[:, ts(m, P)],
                                    op0=ALU.mult, op1=ALU.subtract)
                            tail_update(dst_t, m, k)
                        colsum_batch(dst_t, k)
                        if k > 1:
                            t_prev, t_cur = t_cur, t_prev
                        tgt = t_cur
                        if abs(cg[k]) > 1e-7:
                            nc.vector.scalar_tensor_tensor(
                                out=y1t[:], in0=tgt[:], scalar=float(cg[k]),
                                in1=y1t[:], op0=ALU.mult, op1=ALU.add)
                        if abs(cf[k]) > 1e-7:
                            nc.vector.scalar_tensor_tensor(
                                out=y2t[:], in0=tgt[:], scalar=float(cf[k]),
                                in1=y2t[:], op0=ALU.mult, op1=ALU.add)

                    # wa[:, i] = W_i[my cols, :] @ a1 ; biasa8 = (bias @ a1)/8
                    watmp = aw.tile([P, F], fp16, tag="watmp")
                    wa32 = aw.tile([P, 4], f32, tag="wa32")
                    for i in range(3):
                        nc.vector.tensor_tensor(out=watmp[:], in0=wsl[i][:],
                                                in1=a1B[:], op=ALU.mult)
                        nc.vector.reduce_sum(wa32[:, i:i + 1], watmp[:],
                                             axis=mybir.AxisListType.X)
                    nc.vector.tensor_copy(wa[:, :3], wa32[:, :3])
                    batmp = aw.tile([1, F], fp16, tag="batmp")
                    nc.vector.tensor_tensor(out=batmp[:1, :], in0=bias16[:1, :],
                                            in1=a1_16[:1, :], op=ALU.mult)
                    bsum = aw.tile([1, 2], f32, tag="bsum")
                    nc.vector.reduce_sum(bsum[:1, 0:1], batmp[:1, :],
                                         axis=mybir.AxisListType.X)
                    nc.vector.tensor_scalar(out=biasa8[:1, 0:1],
                                            in0=bsum[:1, 0:1],
                                            scalar1=1.0 / C, scalar2=None,
                                            op0=ALU.mult)
                    # y_i = D^-1/2 y_i~ + addback*sqrt(d)(u^T h); write y^T
                    # blocks straight into the A2A layout (no phase-B
                    # transposes), and accumulate the alpha partials
                    # alpha_c = h_c@wa1 + y1_c@wa2 + y2_c@wa3 + bias.a1/8
                    y16 = v_sc
                    ytmp = tn_tmp  # tn_tmp is dead after the h staging
                    ywide = [[aw.tile([P, R], fp16, name=f"yw_{h}_{b}",
                                      tag=f"yw_{h}_{b}") for b in range(2)]
                             for h in range(2)]
                    acol = aw.tile([P, NT], f32, tag="acol")
                    for (yt, lrow, half) in ((y1t, sqd_row16, 0),
                                             (y2t, sqd_row_e4, 1)):
                        for m in range(NT):
                            ps_m = ps_a.tile([P, P], f32, space="PSUM", tag="psm")
                            nc.tensor.matmul(ps_m[:], lrow[:1, ts(m, P)],
                                             uh_row[:1, :], start=True, stop=True)
                            nc.scalar.activation(ytmp[:, ts(m, P)],
                                                 yt[:, ts(m, P)], AF.Copy,
                                                 scale=dinv[:, m:m + 1])
                            nc.vector.tensor_tensor(out=y16[:, ts(m, P)],
                                                    in0=ytmp[:, ts(m, P)],
                                                    in1=ps_m[:], op=ALU.add)
                            ps_yt = ps_set.tile([P, P], fp16, space="PSUM",
                                                tag=("ytp" if m % 2 == 0
                                                     else "ytp2"),
                                                name=f"yt_{m}_{half}")
                            nc.tensor.transpose(ps_yt[:], y16[:, ts(m, P)],
                                                ident[:])
                            yw = ywide[half][(m // 2) % 2]
                            nc.vector.tensor_copy(
                                yw[:, (m % 2) * P:(m % 2 + 1) * P], ps_yt[:])
                            if m % 2 == 1:
                                r0 = (m // 2) * R + half * P
                                nc.sync.dma_start(out=y12T[r0:r0 + P, :],
                                                  in_=yw[:])
                            ps_ab = ps_set.tile([P, 1], f32, space="PSUM",
                                                tag="pab", name=f"pab_{m}_{half}")
                            ybs = yw[:, (m % 2) * P:(m % 2 + 1) * P]
                            if half == 0:
                                nc.tensor.matmul(ps_ab[:, :1], hcolT[:, ts(m, P)],
                                                 wa[:, 0:1], start=True,
                                                 stop=False)
                                nc.tensor.matmul(ps_ab[:, :1], ybs, wa[:, 1:2],
                                                 start=False, stop=False)
                                nc.tensor.matmul(ps_ab[:, :1], ones_r16[:1, :],
                                                 biasa8[:1, 0:1], start=False,
                                                 stop=True)
                                nc.vector.tensor_copy(acol[:, m:m + 1],
                                                      ps_ab[:, :1])
                            else:
                                nc.tensor.matmul(ps_ab[:, :1], ybs, wa[:, 2:3],
                                                 start=True, stop=True)
                                nc.vector.tensor_tensor(out=acol[:, m:m + 1],
                                                        in0=acol[:, m:m + 1],
                                                        in1=ps_ab[:, :1],
                                                        op=ALU.add)
                    # acol [P, NT] -> node-flat [N, 1] via transpose + one DMA
                    ps_at = ps_set.tile([NT, P], f32, space="PSUM", tag="rowt",
                                        name="ps_at")
                    nc.tensor.transpose(ps_at[:NT, :], acol[:, :NT], id32[:])
                    acolT = aw.tile([NT, P], f32, tag="acolT")
                    nc.vector.tensor_copy(acolT[:NT, :], ps_at[:NT, :])
                    nc.sync.dma_start(out=ab_part[0:N, 0:1], in_=acolT[:NT, :])

                    _scA.__exit__(None, None, None)
                    _scC1 = nc.named_scope("a2a"); _scC1.__enter__()
                    with tc.high_priority():
                        nc.gpsimd.collective_compute(
                            "AllToAll", ALU.bypass, ins=[y12T[:]],
                            outs=[y12x[:]], replica_groups=rgroups)
                        nc.gpsimd.collective_compute(
                            "AllReduce", ALU.add, ins=[ab_part[:]],
                            outs=[abg[:]], replica_groups=rgroups)
                    _scC1.__exit__(None, None, None)

                # =====================================================
                # Phase B: z rows = h@W1 + y1@W2 + y2@W3 + bias
                # (pure matmuls; h-part runs during the A2A)
                # =====================================================
                with (
                    tc.tile_pool(name="bwork", bufs=1) as bw,
                    tc.tile_pool(name="ps_b", bufs=1, space="PSUM") as ps_b,
                ):
                    _scB = nc.named_scope("phaseB"); _scB.__enter__()
                    w_sb = [[bw.tile([P, F], fp16, name=f"w{i}_{k}",
                             tag=f"w{i}_{k}") for k in range(KT)]
                            for i in range(3)]
                    hT_sb = [bw.tile([P, R], fp16, name=f"hT_{k}",
                             tag=f"hT_{k}") for k in range(KT)]
                    for k in range(KT):
                        nc.gpsimd.dma_start(out=w_sb[0][k][:],
                                            in_=d_w[0][ts(k, P), :])
                    for k in range(KT):
                        nc.gpsimd.dma_start(out=hT_sb[k][:],
                                            in_=d_hrowT[ts(k, P), :])
                    for i in (1, 2):
                        for k in range(KT):
                            nc.gpsimd.dma_start(out=w_sb[i][k][:],
                                                in_=d_w[i][ts(k, P), :])
                    ps_h = [[ps_b.tile([P, 512], f32, space="PSUM",
                                       name=f"h{blk}{chunk}", tag=f"h{blk}{chunk}")
                              for chunk in range(2)] for blk in range(2)]
                    ps_y = [[ps_b.tile([P, 512], f32, space="PSUM",
                                       name=f"y{blk}{chunk}", tag=f"y{blk}{chunk}")
                              for chunk in range(2)] for blk in range(2)]
                    # bias + h@W1: contiguous groups, independent of the A2As
                    for blk in range(2):
                        for chunk in range(2):
                            ph = ps_h[blk][chunk]
                            nc.tensor.matmul(ph[:], ones_r16[:1, :],
                                             bias16[:1, ts(chunk, 512)],
                                             start=True, stop=False)
                            for k in range(KT):
                                nc.tensor.matmul(
                                    ph[:], hT_sb[k][:, ts(blk, P)],
                                    w_sb[0][k][:, ts(chunk, 512)],
                                    start=False, stop=(k == KT - 1))
                    # y part: per-bank contiguous groups (y1 then y2)
                    yx = [[bw.tile([P, R], fp16, name=f"yx_{yi}_{s}",
                                   tag=f"yx_{yi}_{s}") for s in range(C)]
                          for yi in range(2)]
                    for yi in range(2):
                        for s in range(C):
                            q = nc.sync if s % 2 == 0 else nc.gpsimd
                            q.dma_start(
                                out=yx[yi][s][:],
                                in_=y12x[s * R + yi * P:s * R + (yi + 1) * P, :])
                    for blk in range(2):
                        for chunk in range(2):
                            py = ps_y[blk][chunk]
                            for yi in range(2):
                                for s in range(C):
                                    nc.tensor.matmul(
                                        py[:], yx[yi][s][:, ts(blk, P)],
                                        w_sb[1 + yi][s][:, ts(chunk, 512)],
                                        start=(yi == 0 and s == 0),
                                        stop=(yi == 1 and s == C - 1))
                    for blk in range(2):
                        z16 = bw.tile([P, F], fp16, tag=f"z16_{blk}",
                                      name=f"z16_{blk}")
                        for chunk in range(2):
                            nc.scalar.activation(z16[:, ts(chunk, 512)],
                                                 ps_h[blk][chunk][:], AF.Copy)
                            nc.vector.scalar_tensor_tensor(
                                out=z16[:, ts(chunk, 512)],
                                in0=ps_y[blk][chunk][:], scalar=1.0,
                                in1=z16[:, ts(chunk, 512)],
                                op0=ALU.mult, op1=ALU.add)
                        nc.sync.dma_start(out=z_slice[ts(blk, P), :], in_=z16[:])
                        abtmp = bw.tile([P, F], fp16, tag=f"abtmp_{blk}",
                                        name=f"abtmp_{blk}")
                        nc.vector.tensor_tensor(out=abtmp[:], in0=z16[:],
                                                in1=a2B[:], op=ALU.mult)
                        nc.vector.reduce_sum(beta_rows[blk][:, 0:1], abtmp[:],
                                             axis=mybir.AxisListType.X)

                    _scB.__exit__(None, None, None)
                    _scC2 = nc.named_scope("ags"); _scC2.__enter__()
                    with tc.high_priority():
                        nc.gpsimd.collective_compute(
                            "AllGather", ALU.bypass, ins=[z_slice[:]],
                            outs=[zg[:]], replica_groups=rgroups)
                    _scC2.__exit__(None, None, None)

            # =========================================================
            # Edge phase (row-sharded dense layered softmax).
            # Softmax chain overlaps the z AllGather; z-dependent work last.
            # =========================================================
            with (
                tc.tile_pool(name="edge", bufs=1) as ep,
                tc.tile_pool(name="edge2", bufs=2) as ep2,
                tc.tile_pool(name="ps_e", bufs=2, space="PSUM") as ps_e,
            ):
                _scE = nc.named_scope("edge"); _scE.__enter__()
                # sync queue: al_row (waits ab-AG) BEFORE z_sb (waits z-AG)
                al_row = ep.tile([1, N], f32, tag="al_row")
                nc.sync.dma_start(out=al_row[:1, :], in_=abg[0:N, :1])
                # alo gathers (need only the alpha AllReduce)
                alo = [ep.tile([P, J_OV], f32, name=f"alo_{b}", tag=f"alo_{b}") for b in range(2)]
                for blk in range(2):
                    for j in range(J_OV):
                        nc.gpsimd.indirect_dma_start(
                            out=alo[blk][:, j:j + 1], out_offset=None,
                            in_=abg[:],
                            in_offset=bass.IndirectOffsetOnAxis(
                                ap=aoff_t[blk][:, j:j + 1], axis=0))
                # z tiles split across both DMA queues; zo gathers last
                z_sb = [ep.tile([P, F], fp16, name=f"z_{t}", tag=f"z_{t}")
                        for t in range(NT)]
                for t in range(11):
                    q = nc.sync if t % 2 == 0 else nc.gpsimd
                    q.dma_start(out=z_sb[t][:], in_=zg[ts(t, P), :])
                zo_t = [[ep.tile([P, F], fp16, name=f"zo_{blk}_{j}",
                                 tag=f"zo_{blk}_{j}") for j in range(J_OV)]
                        for blk in range(2)]
                for blk in range(2):
                    for j in range(J_OV):
                        nc.gpsimd.indirect_dma_start(
                            out=zo_t[blk][j][:], out_offset=None, in_=zg[:],
                            in_offset=bass.IndirectOffsetOnAxis(
                                ap=zoff_t[blk][:, j:j + 1], axis=0))

                # vector: xp (no deps — runs during the AGs)
                xp = [ep.tile([P, W], fp16, name=f"xp_{b}", tag=f"xp_{b}") for b in range(2)]
                for blk in range(2):
                    nc.vector.tensor_scalar(out=xp[blk][:], in0=E1s[blk][:],
                                            scalar1=v01b[:, 1:2], scalar2=None,
                                            op0=ALU.mult)
                    nc.vector.scalar_tensor_tensor(out=xp[blk][:],
                                                   in0=E0s[blk][:],
                                                   scalar=v01b[:, 0:1],
                                                   in1=xp[blk][:],
                                                   op0=ALU.mult, op1=ALU.add)

                # alB broadcast (needs ab-AG)
                al_row16 = ep.tile([1, N], fp16, tag="al_row16")
                nc.vector.tensor_copy(al_row16[:1, :], al_row[:1, :])
                alB = ep.tile([P, N], fp16, tag="alB")
                for chunk in range(N // 512):
                    ps_bb = ps_e.tile([P, 512], f32, space="PSUM", tag="bc")
                    nc.tensor.matmul(ps_bb[:], ones_r16[:1, :],
                                     al_row16[:1, ts(chunk, 512)],
                                     start=True, stop=True)
                    nc.scalar.activation(alB[:, ts(chunk, 512)], ps_bb[:],
                                         AF.Copy)

                # loop 1: softmax build per blk (no z dependence)
                pmat = [ep.tile([P, W], fp16, name=f"pmat_{b}", tag=f"pmat_{b}") for b in range(2)]
                denom = [ep.tile([P, 1], f32, name=f"denom_{b}", tag=f"denom_{b}") for b in range(2)]
                qqT = [ep.tile([2, P], fp16, name=f"qqT_{b}", tag=f"qqT_{b}") for b in range(2)]
                PT = [ep.tile([P, N], fp16, name=f"PT_{b}", tag=f"PT_{b}") for b in range(2)]
                for blk in range(2):
                    beta_blk = beta_rows[blk][:, 0:1]
                    alo_b = ep2.tile([P, J_OV], f32, tag="alo_b")
                    nc.vector.tensor_scalar(out=alo_b[:], in0=alo[blk][:],
                                            scalar1=beta_blk, scalar2=None,
                                            op0=ALU.add)
                    x1 = ep2.tile([P, W], fp16, tag="x1")
                    nc.vector.scalar_tensor_tensor(out=x1[:, 0:N],
                                                   in0=xp[blk][:, 0:N],
                                                   scalar=beta_blk, in1=alB[:],
                                                   op0=ALU.add, op1=ALU.add)
                    nc.vector.tensor_copy(x1[:, N:W], xp[blk][:, N:W])
                    nc.vector.tensor_tensor(out=x1[:, N:N + J_OV],
                                            in0=xp[blk][:, N:N + J_OV],
                                            in1=alo_b[:], op=ALU.add)
                    nc.vector.scalar_tensor_tensor(out=x1[:], in0=x1[:],
                                                   scalar=0.01, in1=x1[:],
                                                   op0=ALU.mult, op1=ALU.max)
                    xm = ep2.tile([P, W], f32, tag="xm")
                    nc.vector.scalar_tensor_tensor(out=xm[:], in0=Ms[blk][:],
                                                   scalar=BIG, in1=x1[:],
                                                   op0=ALU.mult, op1=ALU.add)
                    mx = ep2.tile([P, 1], f32, tag="mx")
                    nc.vector.reduce_max(mx[:], xm[:],
                                         axis=mybir.AxisListType.X)
                    negmx = ep2.tile([P, 1], f32, tag="negmx")
                    nc.vector.tensor_scalar(out=negmx[:], in0=mx[:],
                                            scalar1=-1.0, scalar2=None,
                                            op0=ALU.mult)
                    nc.scalar.activation(pmat[blk][:], xm[:], AF.Exp,
                                         bias=negmx[:, :1],
                                         accum_out=denom[blk][:, :1])
                    s01 = ep2.tile([P, 2], f32, tag="s01")
                    x2 = ep2.tile([P, W], fp16, tag="x2")
                    for (j, Es) in ((0, E0s[blk]), (1, E1s[blk])):
                        nc.vector.scalar_tensor_tensor(
                            out=x2[:], in0=pmat[blk][:], scalar=1.0, in1=Es[:],
                            op0=ALU.mult, op1=ALU.mult,
                            accum_out=s01[:, j:j + 1])
                    q01 = ep2.tile([P, 2], fp16, tag="q01")
                    qtmp = ep2.tile([P, 1], f32, tag="qtmp")
                    for (j, ca, cb) in ((0, ewb[:, 0:1], ewb[:, 1:2]),
                                        (1, ewb[:, 2:3], ewb[:, 3:4])):
                        nc.vector.tensor_scalar(out=qtmp[:], in0=s01[:, 0:1],
                                                scalar1=ca[:, :1], scalar2=None,
                                                op0=ALU.mult)
                        nc.vector.scalar_tensor_tensor(out=q01[:, j:j + 1],
                                                       in0=s01[:, 1:2],
                                                       scalar=cb[:, :1],
                                                       in1=qtmp[:],
                                                       op0=ALU.mult, op1=ALU.add)
                    ps_q = ps_e.tile([P, P], fp16, space="PSUM", tag="tp")
                    nc.tensor.transpose(ps_q[:2, :], q01[:], ident[:])
                    nc.vector.tensor_copy(qqT[blk][:2, :], ps_q[:2, :])
                    for t in range(NT):
                        ps_t = ps_e.tile([P, P], fp16, space="PSUM", tag="tp")
                        nc.tensor.transpose(ps_t[:], pmat[blk][:, ts(t, P)],
                                            ident[:])
                        nc.vector.tensor_copy(PT[blk][:, ts(t, P)], ps_t[:])

                # last z tiles ride the scalar queue once exp is done
                for t in range(11, NT):
                    nc.scalar.dma_start(out=z_sb[t][:], in_=zg[ts(t, P), :])

                # loop 2: z-dependent matmuls + overflow + output
                for blk in range(2):
                    rows = slice(blk * P, (blk + 1) * P)
                    out_sb = ep2.tile([P, F], fp16, tag="out_sb")
                    for chunk in range(2):
                        ps_o = ps_e.tile([P, 512], f32, space="PSUM", tag="pso")
                        nc.tensor.matmul(ps_o[:], qqT[blk][:2, :],
                                         e2nT[:2, ts(chunk, 512)],
                                         start=True, stop=False)
                        for t in range(NT):
                            nc.tensor.matmul(ps_o[:], PT[blk][:, ts(t, P)],
                                             z_sb[t][:, ts(chunk, 512)],
                                             start=False, stop=(t == NT - 1))
                        nc.vector.tensor_copy(out_sb[:, ts(chunk, 512)],
                                              ps_o[:])

                    po32 = ep2.tile([P, J_OV], f32, tag="po32")
                    nc.vector.tensor_copy(po32[:], pmat[blk][:, N:N + J_OV])
                    for j in range(J_OV):
                        nc.vector.scalar_tensor_tensor(
                            out=out_sb[:], in0=zo_t[blk][j][:],
                            scalar=po32[:, j:j + 1], in1=out_sb[:],
                            op0=ALU.mult, op1=ALU.add)

                    recipd = ep2.tile([P, 1], f32, tag="recipd")
                    nc.vector.reciprocal(recipd[:], denom[blk][:])
                    out_f = ep2.tile([P, F], f32, tag="out_f")
                    nc.scalar.activation(out_f[:], out_sb[:], AF.Copy,
                                         scale=recipd[:, :1])
                    nc.sync.dma_start(out=d_out[rows, :], in_=out_f[:])
                _scE.__exit__(None, None, None)

    nc.compile()
    return nc


_PROGRAM_CACHE = {}


def kernel(**inputs):
    h = np.asarray(inputs["h"], np.float32)
    e = np.asarray(inputs["e"], np.float32)
    adj = np.asarray(inputs["adj"], np.float32)
    src = np.asarray(inputs["src"])
    dst = np.asarray(inputs["dst"])
    weight = np.asarray(inputs["weight"], np.float32)
    weight2 = np.asarray(inputs["weight2"], np.float32)
    weight3 = np.asarray(inputs["weight3"], np.float32)
    bias = np.asarray(inputs["bias"], np.float32)
    attn_w = np.asarray(inputs["attn_w"], np.float32)
    edge_w = np.asarray(inputs["edge_w"], np.float32)
    e2n_w = np.asarray(inputs["e2n_w"], np.float32)

    halves, J0, ov, J_OV = _host_prep(e, src, dst)
    e0o, e1o, mo, aoff, zoff = ov

    key = (J0, J_OV)
    if key not in _PROGRAM_CACHE:
        _PROGRAM_CACHE[key] = _build_program(J0, J_OV)
    nc = _PROGRAM_CACHE[key]

    adj16 = adj.astype(np.float16)
    h16 = h.astype(np.float16)
    w16 = [weight[0].astype(np.float16), weight2[0].astype(np.float16),
           weight3[0].astype(np.float16)]

    in_maps = []
    for c in range(C):
        rows = slice(c * R, (c + 1) * R)
        m = {
            "adj": adj16,
            "hcol": np.ascontiguousarray(h16[:, c * COLS:(c + 1) * COLS]),
            "hrowT": np.ascontiguousarray(h16[rows, :].T),
            "hcolT": np.ascontiguousarray(h16[:, c * COLS:(c + 1) * COLS].T),
            "wsl": np.ascontiguousarray(np.concatenate(
                [w16[i][c * COLS:(c + 1) * COLS, :] for i in range(3)])),
            "w1": w16[0], "w2": w16[1], "w3": w16[2],
            "biasv": bias.reshape(1, F),
            "attnw": attn_w.reshape(1, 2 * F + 2),
            "edgew": edge_w,
            "e2nw": e2n_w,
            "e0o": np.ascontiguousarray(e0o[rows]).astype(np.float16),
            "e1o": np.ascontiguousarray(e1o[rows]).astype(np.float16),
            "mo": np.ascontiguousarray(mo[rows]).astype(np.float16),
            "aoff": np.ascontiguousarray(aoff[rows]),
            "zoff": np.ascontiguousarray(zoff[rows]),
        }
        for hf in (0, 1):
            idx_arr, e0_arr, e1_arr = halves[hf]
            m[f"idx0{hf}"] = np.ascontiguousarray(idx_arr[rows])
            m[f"e0h{hf}"] = np.ascontiguousarray(e0_arr[rows]).astype(np.float16)
            m[f"e1h{hf}"] = np.ascontiguousarray(e1_arr[rows]).astype(np.float16)
        in_maps.append(m)

    import os
    trace = bool(os.environ.get("BASS_GNN_TRACE"))
    res = run_bass_kernel_spmd(nc, in_maps, core_ids=list(range(C)),
                               trace=trace)
    if trace:
        kernel.last_results = res
    out = np.empty((N, F), np.float32)
    for c in range(C):
        out[c * R:(c + 1) * R] = res.results[c]["out_rows"]
    return out


if __name__ == "__main__":
    D = np.load("/tmp/refdata.npz")
    inp = {k: D[k] for k in D.files if k != "expected"}
    out = kernel(**inp)
    exp = D["expected"]
    rel = np.linalg.norm(out - exp) / np.linalg.norm(exp)
    print("rel err:", rel)
